# revision 1
# baseline (speedup 1.0000x reference)
"""GRU cell (AnotherGRUCell) on 8 TRN2 NeuronCores.

Strategy: pure data-parallel over batch (8192 rows -> 1024 rows/core),
weights replicated. No collectives.

All on-chip compute is done in TRANSPOSED layout (units on the partition
axis, batch on the free axis):
  - matmul out[n, m] = sum_k W[k, n] * xT[k, m], with the weight tile as
    the stationary operand (lhsT) and xT/hT/rhT as the moving operand.
  - the r/u gate GEMMs x@Wi[:, :2u] + h@Wh[:, :2u] fuse into ONE 32-ktile
    PSUM accumulation over the concatenated operand [xT; hT].
  - the candidate GEMM x@Wi3 + (r*h)@Wh3 similarly accumulates over
    [xT; rhT]; rhT = sigmoid(gates) * hT is produced by ScalarE+VectorE
    already in the [k_part, m_free] layout the matmul needs -> zero
    on-chip transposes.
  - bias is per-partition in this layout, folded into the ScalarE
    activation (sigmoid/tanh) that reads PSUM directly.

The first gate pair is block-interleaved over the k loop so each
freshly-DMA'd x/h tile feeds 4 back-to-back matmuls (2 gate col-tiles x
2 batch chunks) into 4 PSUM banks, hiding the startup input-load
latency behind PE work; steady state interleaves the 2 batch chunks so
consecutive matmuls share the stationary weight tile.

Host side pre-transposes the x/h shards, packs weights into per-column-
tile slabs, casts to bf16, and transposes the [2048, 1024] per-core
output back to [1024, 2048].
"""

import numpy as np
import ml_dtypes

import concourse.bacc as bacc
import concourse.tile as tile
import concourse.mybir as mybir
from concourse.bass_utils import run_bass_kernel_spmd

N_CORES = 8
UNITS = 2048
IN_DIM = 2048
BATCH = 8192
B_LOC = BATCH // N_CORES  # 1024 batch rows per core

P = 128
KT_X = IN_DIM // P           # 16 k-tiles of x
KT_H = UNITS // P            # 16 k-tiles of h
KT = KT_X + KT_H             # 32 contraction k-tiles for [x; h]
NT_G = (2 * UNITS) // P      # 32 gate col-tiles (r: 0..15, u: 16..31)
NT_C = UNITS // P            # 16 candidate col-tiles
M_CHUNK = 512
MC = B_LOC // M_CHUNK        # 2 moving chunks per core

BF16 = mybir.dt.bfloat16
F32 = mybir.dt.float32
NP_BF16 = ml_dtypes.bfloat16

_CACHED_NC = None

# test.py sets TRACE=True to capture the NTFF profile (exec_time_ns +
# perfetto trace); the graded path leaves it off. LAST_RESULTS holds the
# BassKernelResults of the most recent run.
TRACE = False
LAST_RESULTS = None


def _build():
    nc = bacc.Bacc("TRN2", target_bir_lowering=False, debug=False)

    xT = nc.dram_tensor("xT", [KT_X, P, B_LOC], BF16, kind="ExternalInput")
    hT = nc.dram_tensor("hT", [KT_H, P, B_LOC], BF16, kind="ExternalInput")
    # Weights arrive packed in PAIRS of col-tiles: [pair, 128, 2*KT*128],
    # so each pair is one DMA -> one first-use wait on the PE queue.
    w_g = nc.dram_tensor("w_g", [NT_G // 2, P, 2 * KT * P], BF16,
                         kind="ExternalInput")
    w_c = nc.dram_tensor("w_c", [NT_C // 2, P, 2 * KT * P], BF16,
                         kind="ExternalInput")
    # biases transposed: one [128, n_tiles] tensor per gate set -> 1 DMA each
    b_g = nc.dram_tensor("b_g", [P, NT_G], F32, kind="ExternalInput")
    b_c = nc.dram_tensor("b_c", [P, NT_C], F32, kind="ExternalInput")
    out = nc.dram_tensor("out", [NT_C, P, B_LOC], F32, kind="ExternalOutput")

    SIG = mybir.ActivationFunctionType.Sigmoid
    TANH = mybir.ActivationFunctionType.Tanh

    with tile.TileContext(nc) as tc:
        with (
            tc.tile_pool(name="resident", bufs=1) as res,
            tc.tile_pool(name="wslab", bufs=4) as wp,
            tc.tile_pool(name="psum", bufs=8, space="PSUM") as pp,
            tc.tile_pool(name="stage", bufs=2) as sp,
            tc.tile_pool(name="bias", bufs=1) as bp,
        ):
            x_tiles = [
                res.tile([P, B_LOC], BF16, tag=f"x{j}", name=f"x{j}")
                for j in range(KT_X)
            ]
            h_tiles = [
                res.tile([P, B_LOC], BF16, tag=f"h{j}", name=f"h{j}")
                for j in range(KT_H)
            ]
            rh_tiles = [
                res.tile([P, B_LOC], BF16, tag=f"rh{j}", name=f"rh{j}")
                for j in range(KT_H)
            ]
            u_tiles = [
                res.tile([P, B_LOC], BF16, tag=f"u{j}", name=f"u{j}")
                for j in range(NT_C)
            ]

            # PE warm-up: the HAM clock gate holds the PE at 1.2 GHz until
            # it has been busy ~3.4us. The first real matmul can't start
            # before ~11us (preamble + first input DMA), so 8 dummy
            # matmuls (~3.4us at cold rate) fill the 7..10.5us window and
            # un-throttle the PE right before real data lands. Sized so
            # they END before the first operands arrive.
            warm_src = sp.tile(
                [P, M_CHUNK], BF16, tag="warm", name="warm_src", bufs=1
            )
            nc.gpsimd.memset(warm_src[:], 0.0)
            warm_ps = pp.tile([P, M_CHUNK], F32, tag="psum", name="warm_ps")
            for w in range(8):
                nc.tensor.matmul(
                    warm_ps[:],
                    warm_src[:, :P],
                    warm_src[:],
                    start=(w == 0),
                    stop=(w == 7),
                )

            # Startup DMAs in exact consumption order of the first gate
            # pair, interleaved across both HWDGE rings. The rings drain
            # roughly FIFO at HBM rate, so block jb's operands (weight
            # chunk jb/8 of both slabs + src tiles jb..jb+7) are emitted
            # right before the block needs them.
            # Graduated chunk sizes: a tiny first chunk so the very first
            # matmul's dependencies are only ~0.6MB of DMA.
            CHUNKS = [2, 6, 8, 8, 8]
            CB = [0, 2, 8, 16, 24, 32]  # chunk k-tile boundaries
            ws_first = [[None] * len(CHUNKS) for _ in range(2)]  # [t][chunk]
            src_dma = {}  # j -> (engine, dst tile, src ap)
            for j in range(KT_X):
                eng = nc.sync if j % 2 == 0 else nc.scalar
                src_dma[j] = (eng, x_tiles[j], xT[j, :, :])
            for j in range(KT_H):
                eng = nc.scalar if j % 2 == 0 else nc.sync
                src_dma[KT_X + j] = (eng, h_tiles[j], hT[j, :, :])
            for c, cw in enumerate(CHUNKS):
                for t in range(2):
                    w = wp.tile(
                        [P, cw * P], BF16, tag=f"wg{t}_{c}", name=f"wg{t}_{c}",
                        bufs=1,
                    )
                    (nc.sync if t == 0 else nc.scalar).dma_start(
                        w[:],
                        w_g[0, :, (t * KT + CB[c]) * P:(t * KT + CB[c + 1]) * P],
                    )
                    ws_first[t][c] = w
                for j in range(CB[c], CB[c + 1]):
                    eng, dst, src = src_dma[j]
                    if c == 0:
                        # First block: land x0/x1 via the SWDGE queue, in
                        # parallel with the weight chunks on the two HWDGE
                        # rings, so the very first matmuls unblock sooner.
                        eng = nc.gpsimd
                    eng.dma_start(dst[:], src)

            bg_all = bp.tile([P, NT_G], F32, tag="bg", name="bg_all")
            nc.sync.dma_start(bg_all[:], b_g[:, :])
            bc_all = bp.tile([P, NT_C], F32, tag="bc", name="bc_all")
            nc.scalar.dma_start(bc_all[:], b_c[:, :])

            gate_src = x_tiles + h_tiles
            cand_src = x_tiles + rh_tiles

            def act_sig(t, m, ps):
                """sigmoid(psum + b) -> rh (r gates, premultiplied by h) or u."""
                ms = slice(m * M_CHUNK, (m + 1) * M_CHUNK)
                if t < NT_C:
                    rt = sp.tile([P, M_CHUNK], BF16, tag="rtmp", name=f"r{t}_{m}")
                    nc.scalar.activation(rt[:], ps[:], SIG, bias=bg_all[:, t:t + 1])
                    nc.vector.tensor_mul(rh_tiles[t][:, ms], rt[:], h_tiles[t][:, ms])
                else:
                    nc.scalar.activation(
                        u_tiles[t - NT_C][:, ms], ps[:], SIG,
                        bias=bg_all[:, t:t + 1],
                    )

            # Phase 1: gates; t 0..15 -> r, 16..31 -> u.
            #
            # The FIRST pair of gate tiles is block-interleaved (4 psum
            # groups, one block per weight chunk) so the PE has 4 matmuls
            # to run per freshly-arrived x/h tile during the startup
            # loads instead of stalling on the whole operand set.
            t0_groups = [(0, 0), (0, 1), (1, 0), (1, 1)]
            pss0 = [
                pp.tile([P, M_CHUNK], F32, tag="psum", name=f"psg0_{i}")
                for i in range(4)
            ]
            for c in range(len(CHUNKS)):
                for i, (t, m) in enumerate(t0_groups):
                    ms = slice(m * M_CHUNK, (m + 1) * M_CHUNK)
                    for j in range(CB[c], CB[c + 1]):
                        wch = ws_first[t][c]
                        jj = j - CB[c]
                        nc.tensor.matmul(
                            pss0[i][:],
                            wch[:, jj * P:(jj + 1) * P],
                            gate_src[j][:, ms],
                            start=(j == 0),
                            stop=(j == KT - 1),
                        )
            for i, (t, m) in enumerate(t0_groups):
                act_sig(t, m, pss0[i])

            # Weight slabs are loaded in PAIRS (two gate col-tiles per DMA):
            # the PE pays one skipped 216ns beat at each new weight tile's
            # first-use semaphore wait, so halving the tile count halves
            # that cost. Within each t the k loop is m-interleaved so
            # consecutive matmuls share the stationary weight tile.
            def slab_pair(w_dram, tp, name):
                ws = wp.tile([P, 2 * KT * P], BF16, tag="wslab", name=name, bufs=2)
                (nc.sync if (tp // 2) % 2 == 0 else nc.scalar).dma_start(
                    ws[:], w_dram[tp // 2, :, :]
                )
                return ws

            def gemm_group(ws, ti, src_tiles, t, act_fn):
                """One m-interleaved accumulation pair for gate col-tile t,
                using half `ti` of the pair slab `ws`."""
                psl = [
                    pp.tile([P, M_CHUNK], F32, tag="psum", name=f"ps{t}_{m}")
                    for m in range(MC)
                ]
                for j in range(KT):
                    off = (ti * KT + j) * P
                    for m in range(MC):
                        ms = slice(m * M_CHUNK, (m + 1) * M_CHUNK)
                        nc.tensor.matmul(
                            psl[m][:],
                            ws[:, off:off + P],
                            src_tiles[j][:, ms],
                            start=(j == 0),
                            stop=(j == KT - 1),
                        )
                for m in range(MC):
                    act_fn(t, m, psl[m])

            for tp in range(2, NT_G, 2):
                ws = slab_pair(w_g, tp, f"wg{tp}")
                for ti, t in enumerate((tp, tp + 1)):
                    gemm_group(ws, ti, gate_src, t, act_sig)

            # Phase 2: candidate GEMM + tanh + output combine
            # h_t = u * (h - cand) + cand
            def cand_epilogue(t, m, mw, ps):
                ms = slice(m * mw, (m + 1) * mw)
                cand = sp.tile([P, mw], F32, tag="cand", name=f"c{t}_{m}")
                nc.scalar.activation(cand[:], ps[:], TANH, bias=bc_all[:, t:t + 1])
                d = sp.tile([P, mw], F32, tag="d", name=f"d{t}_{m}")
                nc.vector.tensor_sub(d[:], h_tiles[t][:, ms], cand[:])
                d2 = sp.tile([P, mw], F32, tag="d2", name=f"d2{t}_{m}")
                nc.vector.tensor_mul(d2[:], u_tiles[t][:, ms], d[:])
                ht = sp.tile([P, mw], F32, tag="ht", name=f"ht{t}_{m}")
                nc.vector.tensor_add(ht[:], d2[:], cand[:])
                nc.sync.dma_start(out[t, :, ms], ht[:])

            def cand_group(ws, ti, t):
                gemm_group(
                    ws, ti, cand_src, t,
                    lambda t, m, ps: cand_epilogue(t, m, M_CHUNK, ps),
                )

            def cand_group_tapered(ws, ti, t):
                # Last tile: narrow sequential chunks so epilogues stagger
                # and the post-final-matmul tail stays short.
                mw = M_CHUNK // 2
                for m in range(B_LOC // mw):
                    ms = slice(m * mw, (m + 1) * mw)
                    ps = pp.tile([P, mw], F32, tag="psum", name=f"psc{t}_{m}")
                    for j in range(KT):
                        off = (ti * KT + j) * P
                        nc.tensor.matmul(
                            ps[:],
                            ws[:, off:off + P],
                            cand_src[j][:, ms],
                            start=(j == 0),
                            stop=(j == KT - 1),
                        )
                    cand_epilogue(t, m, mw, ps)

            for tp in range(0, NT_C, 2):
                ws = slab_pair(w_c, tp, f"wc{tp}")
                for ti, t in enumerate((tp, tp + 1)):
                    if t == NT_C - 1:
                        cand_group_tapered(ws, ti, t)
                    else:
                        cand_group(ws, ti, t)

    nc.compile()
    return nc


def _get_nc():
    global _CACHED_NC
    if _CACHED_NC is None:
        _CACHED_NC = _build()
    return _CACHED_NC


def _pack_w(w):
    """[K, N] fp32 -> [N/128, 128, K] bf16 slab layout:
    slab[t, p, j*128 + c] = w[j*128 + p, t*128 + c]"""
    K, N = w.shape
    a = w.reshape(K // P, P, N // P, P).transpose(2, 1, 0, 3)
    return np.ascontiguousarray(a).astype(NP_BF16).reshape(N // P, P, K)


def kernel(x_t, h_tm1, input_weight, hidden_state_weight, bias):
    x_t = np.asarray(x_t, dtype=np.float32)
    h_tm1 = np.asarray(h_tm1, dtype=np.float32)
    input_weight = np.asarray(input_weight, dtype=np.float32)
    hidden_state_weight = np.asarray(hidden_state_weight, dtype=np.float32)
    bias = np.asarray(bias, dtype=np.float32)

    u = UNITS
    # Gate weights: [x; h] @ [Wi[:, :2u]; Wh[:, :2u]]
    w_gate = np.concatenate(
        [input_weight[:, : 2 * u], hidden_state_weight[:, : 2 * u]], axis=0
    )  # [4096, 4096]
    w_cand = np.concatenate(
        [input_weight[:, 2 * u:], hidden_state_weight[:, 2 * u:]], axis=0
    )  # [4096, 2048]
    def _pair(w):  # [T, 128, K] -> [T/2, 128, 2K]: pairs contiguous per DMA
        T, p, K = w.shape
        return np.ascontiguousarray(
            w.reshape(T // 2, 2, p, K).transpose(0, 2, 1, 3)
        ).reshape(T // 2, p, 2 * K)

    w_g_np = _pair(_pack_w(w_gate))  # [16, 128, 8192] bf16
    w_c_np = _pair(_pack_w(w_cand))  # [8, 128, 8192] bf16
    b_g_np = np.ascontiguousarray(bias[: 2 * u].reshape(NT_G, P).T, dtype=np.float32)
    b_c_np = np.ascontiguousarray(bias[2 * u:].reshape(NT_C, P).T, dtype=np.float32)

    in_maps = []
    for i in range(N_CORES):
        sl = slice(i * B_LOC, (i + 1) * B_LOC)
        xT_np = x_t[sl].T.astype(NP_BF16).reshape(KT_X, P, B_LOC)
        hT_np = h_tm1[sl].T.astype(NP_BF16).reshape(KT_H, P, B_LOC)
        in_maps.append(
            {
                "xT": np.ascontiguousarray(xT_np),
                "hT": np.ascontiguousarray(hT_np),
                "w_g": w_g_np,
                "w_c": w_c_np,
                "b_g": b_g_np,
                "b_c": b_c_np,
            }
        )

    nc = _get_nc()
    res = run_bass_kernel_spmd(
        nc, in_maps, core_ids=list(range(N_CORES)), trace=TRACE
    )
    global LAST_RESULTS
    LAST_RESULTS = res

    h_t = np.empty((BATCH, UNITS), dtype=np.float32)
    for i in range(N_CORES):
        o = np.asarray(res.results[i]["out"], dtype=np.float32)
        h_t[i * B_LOC:(i + 1) * B_LOC] = o.reshape(UNITS, B_LOC).T
    return h_t



# revision 3
# speedup vs baseline: 1.1507x; 1.1507x over previous
"""GRU cell (AnotherGRUCell) on 8 TRN2 NeuronCores.

Strategy: pure data-parallel over batch (8192 rows -> 1024 rows/core),
weights replicated. No collectives.

All on-chip compute is in TRANSPOSED layout (units on the partition axis,
batch on the free axis), as in the bf16 baseline. New here: most of the
GEMM work runs in fp8-e4m3 with perf_mode=DoubleRow (2 k-tiles per PE
instruction), which roughly doubles PE matmul throughput. fp8
quantization noise is steered by a per-unit precision config chosen via
a host-side numpy simulation against the 2e-2 rel-err gate:

  - r gates (cols 0..15):     x@Wr + h@Wr fully fp8 (error is attenuated
                              through r*h -> cand -> (1-u) weighting)
  - u gates (cols 16..31):    first N8_XU/N8_HU k-tiles of x/h in fp8,
                              rest bf16 (u multiplies h directly in the
                              output, so u noise is expensive)
  - cand (r*h)@Wh3:           fully fp8 (attenuated like r)
  - cand x@Wi3:               bf16 (tanh pre-act noise is expensive)

All weights (both dtypes) are pre-scaled by S_W=32 on the host so fp8
sees a ~unit-std distribution, and every PSUM accumulation has one
uniform scale that is divided out for free inside the ScalarE
activation (out = sigmoid/tanh(psum * 1/S_W + bias)).

fp8 x/h/rh operands live in PAIR tiles [128, 2, 1024] so each DoubleRow
matmul gets its required 3D AP [128, 2, free] (pair-dim step % 16 == 0)
while startup DMAs keep per-pair dependency granularity.
"""

import numpy as np
import ml_dtypes

import concourse.bacc as bacc
import concourse.tile as tile
import concourse.mybir as mybir
from concourse.bass_utils import run_bass_kernel_spmd

N_CORES = 8
UNITS = 2048
IN_DIM = 2048
BATCH = 8192
B_LOC = BATCH // N_CORES  # 1024 batch rows per core

P = 128
KT_X = IN_DIM // P           # 16 k-tiles of x
KT_H = UNITS // P            # 16 k-tiles of h
KT = KT_X + KT_H             # 32 contraction k-tiles for [x; h]
NT_G = (2 * UNITS) // P      # 32 gate col-tiles (r: 0..15, u: 16..31)
NT_C = UNITS // P            # 16 candidate col-tiles
M_CHUNK = 512
MC = B_LOC // M_CHUNK        # 2 moving chunks per core

# Precision config: number of leading k-tiles (of 16) computed in fp8
# DoubleRow for the u-gate x/h operands. r gates and (r*h)@Wh3 are fully
# fp8; the candidate x@Wi3 is fully bf16. Must be even (DoubleRow pairs).
N8_XU = 4
N8_HU = 4
S_W = 32.0
S_INV = float(1.0 / S_W)

BF16 = mybir.dt.bfloat16
F32 = mybir.dt.float32
FP8 = mybir.dt.float8e4
NP_BF16 = ml_dtypes.bfloat16
NP_FP8 = ml_dtypes.float8_e4m3  # IEEE-style e4m3, max 240 == TRN FP8_EXP4
DR = mybir.MatmulPerfMode.DoubleRow

_CACHED_NC = None

# test.py sets TRACE=True to capture the NTFF profile (exec_time_ns +
# perfetto trace); the graded path leaves it off. LAST_RESULTS holds the
# BassKernelResults of the most recent run.
TRACE = False
LAST_RESULTS = None


def _build():
    nc = bacc.Bacc("TRN2", target_bir_lowering=False, debug=False)

    # fp8 transposed inputs, one [128, 1024] slice per k-tile
    x8d = nc.dram_tensor("x8", [KT_X, P, B_LOC], FP8, kind="ExternalInput")
    h8d = nc.dram_tensor("h8", [KT_H, P, B_LOC], FP8, kind="ExternalInput")
    # bf16 transposed inputs (u-gate bf16 part, cand x part, epilogues)
    xbd = nc.dram_tensor("xb", [KT_X, P, B_LOC], BF16, kind="ExternalInput")
    hbd = nc.dram_tensor("hb", [KT_H, P, B_LOC], BF16, kind="ExternalInput")
    # weights, pre-scaled by S_W, packed per col-tile as [128p, nkt, 128c]
    n8u = N8_XU + N8_HU
    nbu = KT - n8u
    w_r8 = nc.dram_tensor("w_r8", [NT_C, P, KT, P], FP8, kind="ExternalInput")
    w_u8 = nc.dram_tensor("w_u8", [NT_C, P, n8u, P], FP8, kind="ExternalInput")
    w_ub = nc.dram_tensor("w_ub", [NT_C, P, nbu, P], BF16, kind="ExternalInput")
    w_c8 = nc.dram_tensor("w_c8", [NT_C, P, KT_H, P], FP8, kind="ExternalInput")
    w_cb = nc.dram_tensor("w_cb", [NT_C, P, KT_X, P], BF16, kind="ExternalInput")
    # biases transposed: one [128, n_tiles] tensor per gate set -> 1 DMA each
    b_g = nc.dram_tensor("b_g", [P, NT_G], F32, kind="ExternalInput")
    b_c = nc.dram_tensor("b_c", [P, NT_C], F32, kind="ExternalInput")
    out = nc.dram_tensor("out", [NT_C, P, B_LOC], F32, kind="ExternalOutput")

    SIG = mybir.ActivationFunctionType.Sigmoid
    TANH = mybir.ActivationFunctionType.Tanh

    NPAIR_X = KT_X // 2
    NPAIR_H = KT_H // 2

    with tile.TileContext(nc) as tc:
        with (
            tc.tile_pool(name="resident", bufs=1) as res,
            tc.tile_pool(name="wslab", bufs=2) as wp,
            tc.tile_pool(name="psum", bufs=8, space="PSUM") as pp,
            tc.tile_pool(name="stage", bufs=2) as sp,
            tc.tile_pool(name="bias", bufs=1) as bp,
        ):
            # fp8 pair tiles: [128, 2, 1024]; pair q holds k-tiles 2q, 2q+1
            x8_pairs = [
                res.tile([P, 2, B_LOC], FP8, tag=f"x8{q}", name=f"x8{q}")
                for q in range(NPAIR_X)
            ]
            h8_pairs = [
                res.tile([P, 2, B_LOC], FP8, tag=f"h8{q}", name=f"h8{q}")
                for q in range(NPAIR_H)
            ]
            rh8_pairs = [
                res.tile([P, 2, B_LOC], FP8, tag=f"rh{q}", name=f"rh{q}")
                for q in range(NPAIR_H)
            ]
            # bf16 per-k-tile tiles
            xb_tiles = [
                res.tile([P, B_LOC], BF16, tag=f"xb{j}", name=f"xb{j}")
                for j in range(KT_X)
            ]
            hb_tiles = [
                res.tile([P, B_LOC], BF16, tag=f"hb{j}", name=f"hb{j}")
                for j in range(KT_H)
            ]
            u_tiles = [
                res.tile([P, B_LOC], BF16, tag=f"u{j}", name=f"u{j}")
                for j in range(NT_C)
            ]

            # PE warm-up: the HAM clock gate holds the PE at 1.2 GHz until
            # it has been busy ~3.4us; fill the pre-first-matmul window
            # with dummy matmuls so the PE is un-throttled when real data
            # lands (same trick as the bf16 baseline).
            warm_src = sp.tile(
                [P, M_CHUNK], BF16, tag="warm", name="warm_src", bufs=1
            )
            nc.gpsimd.memset(warm_src[:], 0.0)
            warm_ps = pp.tile([P, M_CHUNK], F32, tag="psum", name="warm_ps")
            for w in range(8):
                nc.tensor.matmul(
                    warm_ps[:],
                    warm_src[:, :P],
                    warm_src[:],
                    start=(w == 0),
                    stop=(w == 7),
                )

            # Startup DMAs in exact consumption order of the first r-gate
            # col-tile pair, interleaved across both HWDGE rings.
            # Graduated chunk sizes (in k-tiles over the 32-long [x; h]
            # sequence); all chunk boundaries are even so DoubleRow pairs
            # never straddle a chunk.
            CHUNKS = [2, 6, 8, 8, 8]
            CB = [0, 2, 8, 16, 24, 32]  # chunk k-tile boundaries
            ws_first = [[None] * len(CHUNKS) for _ in range(2)]  # [t][chunk]
            src_dma = {}  # k-slot -> (engine, dst ap, src ap)
            for j in range(KT_X):
                eng = nc.sync if j % 2 == 0 else nc.scalar
                src_dma[j] = (eng, x8_pairs[j // 2][:, j % 2, :], x8d[j, :, :])
            for j in range(KT_H):
                eng = nc.scalar if j % 2 == 0 else nc.sync
                src_dma[KT_X + j] = (
                    eng, h8_pairs[j // 2][:, j % 2, :], h8d[j, :, :]
                )
            for c, cw in enumerate(CHUNKS):
                for t in range(2):
                    w = wp.tile(
                        [P, cw, P], FP8, tag=f"wr{t}_{c}", name=f"wr{t}_{c}",
                        bufs=1,
                    )
                    (nc.sync if t == 0 else nc.scalar).dma_start(
                        w[:], w_r8[t, :, CB[c]:CB[c + 1], :]
                    )
                    ws_first[t][c] = w
                for j in range(CB[c], CB[c + 1]):
                    eng, dst, src = src_dma[j]
                    if c == 0:
                        # First block via the SWDGE queue, in parallel with
                        # the weight chunks on the two HWDGE rings.
                        eng = nc.gpsimd
                    eng.dma_start(dst, src)

            bg_all = bp.tile([P, NT_G], F32, tag="bg", name="bg_all")
            nc.sync.dma_start(bg_all[:], b_g[:, :])
            bc_all = bp.tile([P, NT_C], F32, tag="bc", name="bc_all")
            nc.scalar.dma_start(bc_all[:], b_c[:, :])

            # bf16 inputs land during the r-gate phase: h first (r-gate
            # epilogues need hb[t] at col-tile t), then x (u phase).
            for j in range(KT_H):
                (nc.sync if j % 2 == 0 else nc.scalar).dma_start(
                    hb_tiles[j][:], hbd[j, :, :]
                )
            for j in range(KT_X):
                (nc.scalar if j % 2 == 0 else nc.sync).dma_start(
                    xb_tiles[j][:], xbd[j, :, :]
                )

            all_pairs = x8_pairs + h8_pairs  # 16 fp8 pair tiles = 32 k-tiles

            def act_r(t, m, ps):
                """r epilogue: rh8[t] = sigmoid(ps/S_W + b) * h  (fp8)."""
                ms = slice(m * M_CHUNK, (m + 1) * M_CHUNK)
                rt = sp.tile([P, M_CHUNK], BF16, tag="rtmp", name=f"r{t}_{m}")
                nc.scalar.activation(
                    rt[:], ps[:], SIG, bias=bg_all[:, t:t + 1], scale=S_INV
                )
                nc.vector.tensor_mul(
                    rh8_pairs[t // 2][:, t % 2, ms], rt[:], hb_tiles[t][:, ms]
                )

            def act_u(t, m, ps):
                ms = slice(m * M_CHUNK, (m + 1) * M_CHUNK)
                nc.scalar.activation(
                    u_tiles[t - NT_C][:, ms], ps[:], SIG,
                    bias=bg_all[:, t:t + 1], scale=S_INV,
                )

            # ---- Phase R: r gates (cols 0..15), fully fp8 DoubleRow ------
            # First col-tile pair is block-interleaved over the startup
            # chunks (4 psum groups) so the PE has matmuls to run per
            # freshly-arrived chunk instead of stalling on the full set.
            t0_groups = [(0, 0), (0, 1), (1, 0), (1, 1)]
            pss0 = [
                pp.tile([P, M_CHUNK], F32, tag="psum", name=f"psg0_{i}")
                for i in range(4)
            ]
            for c in range(len(CHUNKS)):
                q0, q1 = CB[c] // 2, CB[c + 1] // 2
                for i, (t, m) in enumerate(t0_groups):
                    ms = slice(m * M_CHUNK, (m + 1) * M_CHUNK)
                    for qq in range(q0, q1):
                        jj = qq - q0  # pair index within this chunk's slab
                        nc.tensor.matmul(
                            pss0[i][:],
                            ws_first[t][c][:, 2 * jj:2 * jj + 2, :],
                            all_pairs[qq][:, 0:2, ms],
                            start=(qq == 0),
                            stop=(qq == KT // 2 - 1),
                            perf_mode=DR,
                        )
            for i, (t, m) in enumerate(t0_groups):
                act_r(t, m, pss0[i])

            # Steady-state r cols: one fp8 slab [128, 32, 128] per col-tile,
            # m-interleaved so consecutive matmuls share the stationary
            # weight pair (one 256-col LDWEIGHTS per 2 matmuls).
            for t in range(2, NT_C):
                ws = wp.tile([P, KT, P], FP8, tag="wr", name=f"wr{t}")
                (nc.sync if t % 2 == 0 else nc.scalar).dma_start(
                    ws[:], w_r8[t, :, :, :]
                )
                psl = [
                    pp.tile([P, M_CHUNK], F32, tag="psum", name=f"psr{t}_{m}")
                    for m in range(MC)
                ]
                for q in range(KT // 2):
                    for m in range(MC):
                        ms = slice(m * M_CHUNK, (m + 1) * M_CHUNK)
                        nc.tensor.matmul(
                            psl[m][:],
                            ws[:, 2 * q:2 * q + 2, :],
                            all_pairs[q][:, 0:2, ms],
                            start=(q == 0),
                            stop=(q == KT // 2 - 1),
                            perf_mode=DR,
                        )
                for m in range(MC):
                    act_r(t, m, psl[m])

            # ---- Phase U: u gates (cols 16..31), mixed fp8/bf16 ----------
            for u in range(NT_C):
                t = NT_C + u
                w8 = None
                if n8u:
                    w8 = wp.tile([P, n8u, P], FP8, tag="wu8", name=f"wu8_{u}")
                    (nc.sync if u % 2 == 0 else nc.scalar).dma_start(
                        w8[:], w_u8[u, :, :, :]
                    )
                wb = wp.tile([P, nbu, P], BF16, tag="wub", name=f"wub_{u}")
                (nc.scalar if u % 2 == 0 else nc.sync).dma_start(
                    wb[:], w_ub[u, :, :, :]
                )
                psl = [
                    pp.tile([P, M_CHUNK], F32, tag="psum", name=f"psu{t}_{m}")
                    for m in range(MC)
                ]
                n_mm = n8u // 2 + nbu  # total accumulation steps per m
                step = 0
                # fp8 DoubleRow part: x pairs then h pairs
                for q in range(N8_XU // 2):
                    for m in range(MC):
                        ms = slice(m * M_CHUNK, (m + 1) * M_CHUNK)
                        nc.tensor.matmul(
                            psl[m][:],
                            w8[:, 2 * q:2 * q + 2, :],
                            x8_pairs[q][:, 0:2, ms],
                            start=(step == 0),
                            stop=(step == n_mm - 1),
                            perf_mode=DR,
                        )
                    step += 1
                for q in range(N8_HU // 2):
                    off = N8_XU + 2 * q
                    for m in range(MC):
                        ms = slice(m * M_CHUNK, (m + 1) * M_CHUNK)
                        nc.tensor.matmul(
                            psl[m][:],
                            w8[:, off:off + 2, :],
                            h8_pairs[q][:, 0:2, ms],
                            start=(step == 0),
                            stop=(step == n_mm - 1),
                            perf_mode=DR,
                        )
                    step += 1
                # bf16 part: x k-tiles N8_XU..15, then h k-tiles N8_HU..15
                for i, src in enumerate(
                    [xb_tiles[j] for j in range(N8_XU, KT_X)]
                    + [hb_tiles[j] for j in range(N8_HU, KT_H)]
                ):
                    for m in range(MC):
                        ms = slice(m * M_CHUNK, (m + 1) * M_CHUNK)
                        nc.tensor.matmul(
                            psl[m][:],
                            wb[:, i, :],
                            src[:, ms],
                            start=(step == 0),
                            stop=(step == n_mm - 1),
                        )
                    step += 1
                for m in range(MC):
                    act_u(t, m, psl[m])

            # ---- Phase C: candidate + output combine ---------------------
            # psum = (r*h)@Wh3 (fp8 DR) + x@Wi3 (bf16);
            # h_t = u * (h - cand) + cand
            def cand_epilogue(t, m, mw, ps):
                ms = slice(m * mw, (m + 1) * mw)
                cand = sp.tile([P, mw], F32, tag="cand", name=f"c{t}_{m}")
                nc.scalar.activation(
                    cand[:], ps[:], TANH, bias=bc_all[:, t:t + 1], scale=S_INV
                )
                d = sp.tile([P, mw], F32, tag="d", name=f"d{t}_{m}")
                nc.vector.tensor_sub(d[:], hb_tiles[t][:, ms], cand[:])
                d2 = sp.tile([P, mw], F32, tag="d2", name=f"d2{t}_{m}")
                nc.vector.tensor_mul(d2[:], u_tiles[t][:, ms], d[:])
                ht = sp.tile([P, mw], F32, tag="ht", name=f"ht{t}_{m}")
                nc.vector.tensor_add(ht[:], d2[:], cand[:])
                nc.sync.dma_start(out[t, :, ms], ht[:])

            def cand_slabs(t):
                w8 = wp.tile([P, KT_H, P], FP8, tag="wc8", name=f"wc8_{t}")
                (nc.sync if t % 2 == 0 else nc.scalar).dma_start(
                    w8[:], w_c8[t, :, :, :]
                )
                wb = wp.tile([P, KT_X, P], BF16, tag="wcb", name=f"wcb_{t}")
                (nc.scalar if t % 2 == 0 else nc.sync).dma_start(
                    wb[:], w_cb[t, :, :, :]
                )
                return w8, wb

            def cand_accum(w8, wb, ps_of_m, mws):
                """Accumulate candidate GEMM for the m-chunks in ps_of_m
                (dict m -> psum tile), chunk width mws."""
                n_mm = KT_H // 2 + KT_X
                step = 0
                for q in range(KT_H // 2):
                    for m, ps in ps_of_m.items():
                        ms = slice(m * mws, (m + 1) * mws)
                        nc.tensor.matmul(
                            ps[:],
                            w8[:, 2 * q:2 * q + 2, :],
                            rh8_pairs[q][:, 0:2, ms],
                            start=(step == 0),
                            stop=(step == n_mm - 1),
                            perf_mode=DR,
                        )
                    step += 1
                for j in range(KT_X):
                    for m, ps in ps_of_m.items():
                        ms = slice(m * mws, (m + 1) * mws)
                        nc.tensor.matmul(
                            ps[:],
                            wb[:, j, :],
                            xb_tiles[j][:, ms],
                            start=(step == 0),
                            stop=(step == n_mm - 1),
                        )
                    step += 1

            for t in range(NT_C):
                w8, wb = cand_slabs(t)
                if t < NT_C - 1:
                    psl = {
                        m: pp.tile(
                            [P, M_CHUNK], F32, tag="psum", name=f"psc{t}_{m}"
                        )
                        for m in range(MC)
                    }
                    cand_accum(w8, wb, psl, M_CHUNK)
                    for m in range(MC):
                        cand_epilogue(t, m, M_CHUNK, psl[m])
                else:
                    # Last tile: narrow sequential chunks so epilogues
                    # stagger and the post-final-matmul tail stays short.
                    mw = M_CHUNK // 2
                    for m in range(B_LOC // mw):
                        ps = pp.tile(
                            [P, mw], F32, tag="psum", name=f"psc{t}_{m}"
                        )
                        cand_accum(w8, wb, {m: ps}, mw)
                        cand_epilogue(t, m, mw, ps)

    nc.compile()
    return nc


def _get_nc():
    global _CACHED_NC
    if _CACHED_NC is None:
        _CACHED_NC = _build()
    return _CACHED_NC


def _ct_blocks(w):
    """[K, N] -> [N/128 col-tiles, K/128 k-tiles, 128p, 128c] blocks."""
    K, N = w.shape
    return np.ascontiguousarray(
        w.reshape(K // P, P, N // P, P).transpose(2, 0, 1, 3)
    )


def _slab(blocks, ct, sel, np_dtype):
    """Pack k-tiles `sel` of col-tile ct into [128p, len(sel), 128c]."""
    a = blocks[ct][sel]  # [nkt, 128p, 128c]
    return np.ascontiguousarray(a.transpose(1, 0, 2)).astype(np_dtype)


def kernel(x_t, h_tm1, input_weight, hidden_state_weight, bias):
    x_t = np.asarray(x_t, dtype=np.float32)
    h_tm1 = np.asarray(h_tm1, dtype=np.float32)
    input_weight = np.asarray(input_weight, dtype=np.float32)
    hidden_state_weight = np.asarray(hidden_state_weight, dtype=np.float32)
    bias = np.asarray(bias, dtype=np.float32)

    u = UNITS
    # Gate weights: [x; h] @ [Wi[:, :2u]; Wh[:, :2u]], pre-scaled by S_W
    w_gate = np.concatenate(
        [input_weight[:, : 2 * u], hidden_state_weight[:, : 2 * u]], axis=0
    ) * np.float32(S_W)  # [4096, 4096]
    w_cand = np.concatenate(
        [input_weight[:, 2 * u:], hidden_state_weight[:, 2 * u:]], axis=0
    ) * np.float32(S_W)  # [4096, 2048]

    bg = _ct_blocks(w_gate)   # [32 ct, 32 kt, 128, 128]
    bc = _ct_blocks(w_cand)   # [16 ct, 32 kt, 128, 128]

    kt_all = list(range(KT))
    sel_u8 = list(range(N8_XU)) + list(range(KT_X, KT_X + N8_HU))
    sel_ub = list(range(N8_XU, KT_X)) + list(range(KT_X + N8_HU, KT))
    sel_c8 = list(range(KT_X, KT))      # rh k-tiles (h rows of w_cand)
    sel_cb = list(range(KT_X))          # x k-tiles

    w_r8_np = np.stack([_slab(bg, t, kt_all, NP_FP8) for t in range(NT_C)])
    w_u8_np = np.stack(
        [_slab(bg, NT_C + t, sel_u8, NP_FP8) for t in range(NT_C)]
    )
    w_ub_np = np.stack(
        [_slab(bg, NT_C + t, sel_ub, NP_BF16) for t in range(NT_C)]
    )
    w_c8_np = np.stack([_slab(bc, t, sel_c8, NP_FP8) for t in range(NT_C)])
    w_cb_np = np.stack([_slab(bc, t, sel_cb, NP_BF16) for t in range(NT_C)])

    b_g_np = np.ascontiguousarray(
        bias[: 2 * u].reshape(NT_G, P).T, dtype=np.float32
    )
    b_c_np = np.ascontiguousarray(
        bias[2 * u:].reshape(NT_C, P).T, dtype=np.float32
    )

    in_maps = []
    for i in range(N_CORES):
        sl = slice(i * B_LOC, (i + 1) * B_LOC)
        xT = x_t[sl].T  # [2048, 1024] fp32
        hT = h_tm1[sl].T
        in_maps.append(
            {
                "x8": np.ascontiguousarray(
                    xT.astype(NP_FP8).reshape(KT_X, P, B_LOC)
                ),
                "h8": np.ascontiguousarray(
                    hT.astype(NP_FP8).reshape(KT_H, P, B_LOC)
                ),
                "xb": np.ascontiguousarray(
                    xT.astype(NP_BF16).reshape(KT_X, P, B_LOC)
                ),
                "hb": np.ascontiguousarray(
                    hT.astype(NP_BF16).reshape(KT_H, P, B_LOC)
                ),
                "w_r8": w_r8_np,
                "w_u8": w_u8_np,
                "w_ub": w_ub_np,
                "w_c8": w_c8_np,
                "w_cb": w_cb_np,
                "b_g": b_g_np,
                "b_c": b_c_np,
            }
        )

    nc = _get_nc()
    res = run_bass_kernel_spmd(
        nc, in_maps, core_ids=list(range(N_CORES)), trace=TRACE
    )
    global LAST_RESULTS
    LAST_RESULTS = res

    h_t = np.empty((BATCH, UNITS), dtype=np.float32)
    for i in range(N_CORES):
        o = np.asarray(res.results[i]["out"], dtype=np.float32)
        h_t[i * B_LOC:(i + 1) * B_LOC] = o.reshape(UNITS, B_LOC).T
    return h_t


# revision 8
# speedup vs baseline: 1.4725x; 1.2797x over previous
"""GRU cell (AnotherGRUCell) on 8 TRN2 NeuronCores.

Strategy: pure data-parallel over batch (8192 rows -> 1024 rows/core),
weights replicated. No collectives.

All on-chip compute is in TRANSPOSED layout (units on the partition axis,
batch on the free axis), as in the bf16 baseline. New here: most of the
GEMM work runs in fp8-e4m3 with perf_mode=DoubleRow (2 k-tiles per PE
instruction), which roughly doubles PE matmul throughput. fp8
quantization noise is steered by a per-unit precision config chosen via
a host-side numpy simulation against the 2e-2 rel-err gate:

  - r gates (cols 0..15):     x@Wr + h@Wr fully fp8 (error is attenuated
                              through r*h -> cand -> (1-u) weighting)
  - u gates (cols 16..31):    first N8_XU/N8_HU k-tiles of x/h in fp8,
                              rest bf16 (u multiplies h directly in the
                              output, so u noise is expensive)
  - cand (r*h)@Wh3:           fully fp8 (attenuated like r)
  - cand x@Wi3:               bf16 (tanh pre-act noise is expensive)

All weights (both dtypes) are pre-scaled by S_W=32 on the host so fp8
sees a ~unit-std distribution, and every PSUM accumulation has one
uniform scale that is divided out for free inside the ScalarE
activation (out = sigmoid/tanh(psum * 1/S_W + bias)).

fp8 x/h/rh operands live in PAIR tiles [128, 2, 1024] so each DoubleRow
matmul gets its required 3D AP [128, 2, free] (pair-dim step % 16 == 0)
while startup DMAs keep per-pair dependency granularity.
"""

import numpy as np
import ml_dtypes

import concourse.bacc as bacc
import concourse.tile as tile
import concourse.mybir as mybir
from concourse.bass_utils import run_bass_kernel_spmd

N_CORES = 8
UNITS = 2048
IN_DIM = 2048
BATCH = 8192
B_LOC = BATCH // N_CORES  # 1024 batch rows per core

P = 128
KT_X = IN_DIM // P           # 16 k-tiles of x
KT_H = UNITS // P            # 16 k-tiles of h
KT = KT_X + KT_H             # 32 contraction k-tiles for [x; h]
NT_G = (2 * UNITS) // P      # 32 gate col-tiles (r: 0..15, u: 16..31)
NT_C = UNITS // P            # 16 candidate col-tiles
M_CHUNK = 512
MC = B_LOC // M_CHUNK        # 2 moving chunks per core

# Precision config: number of leading k-tiles (of 16) computed in fp8
# DoubleRow for the u-gate x/h operands. r gates and (r*h)@Wh3 are fully
# fp8; the candidate x@Wi3 is fully bf16. Must be even (DoubleRow pairs).
# Host-sim predicted rel err (matches HW to ~5 digits): 4 -> 1.571e-2,
# 8 -> 1.811e-2, 10 -> 1.920e-2 against the 2e-2 gate.
N8_XU = 8
N8_HU = 8
S_W = 32.0
S_INV = float(1.0 / S_W)

BF16 = mybir.dt.bfloat16
F32 = mybir.dt.float32
FP8 = mybir.dt.float8e4
NP_BF16 = ml_dtypes.bfloat16
NP_FP8 = ml_dtypes.float8_e4m3  # IEEE-style e4m3, max 240 == TRN FP8_EXP4
DR = mybir.MatmulPerfMode.DoubleRow

_CACHED_NC = None

# test.py sets TRACE=True to capture the NTFF profile (exec_time_ns +
# perfetto trace); the graded path leaves it off. LAST_RESULTS holds the
# BassKernelResults of the most recent run.
TRACE = False
LAST_RESULTS = None


def _build():
    nc = bacc.Bacc("TRN2", target_bir_lowering=False, debug=False)

    # fp8 transposed inputs, one [128, 1024] slice per k-tile
    x8d = nc.dram_tensor("x8", [KT_X, P, B_LOC], FP8, kind="ExternalInput")
    h8d = nc.dram_tensor("h8", [KT_H, P, B_LOC], FP8, kind="ExternalInput")
    # bf16 transposed inputs (u-gate bf16 part, cand x part, epilogues)
    xbd = nc.dram_tensor("xb", [KT_X, P, B_LOC], BF16, kind="ExternalInput")
    hbd = nc.dram_tensor("hb", [KT_H, P, B_LOC], BF16, kind="ExternalInput")
    # weights, pre-scaled by S_W, packed per col-tile as [128p, nkt, 128c]
    n8u = N8_XU + N8_HU
    nbu = KT - n8u
    w_r8 = nc.dram_tensor("w_r8", [NT_C, P, KT, P], FP8, kind="ExternalInput")
    w_u8 = nc.dram_tensor("w_u8", [NT_C, P, n8u, P], FP8, kind="ExternalInput")
    w_ub = nc.dram_tensor("w_ub", [NT_C, P, nbu, P], BF16, kind="ExternalInput")
    w_c8 = nc.dram_tensor("w_c8", [NT_C, P, KT_H, P], FP8, kind="ExternalInput")
    w_cb = nc.dram_tensor("w_cb", [NT_C, P, KT_X, P], BF16, kind="ExternalInput")
    # biases transposed: one [128, n_tiles] tensor per gate set -> 1 DMA each
    b_g = nc.dram_tensor("b_g", [P, NT_G], F32, kind="ExternalInput")
    b_c = nc.dram_tensor("b_c", [P, NT_C], F32, kind="ExternalInput")
    out = nc.dram_tensor("out", [NT_C, P, B_LOC], F32, kind="ExternalOutput")

    SIG = mybir.ActivationFunctionType.Sigmoid
    TANH = mybir.ActivationFunctionType.Tanh

    NPAIR_X = KT_X // 2
    NPAIR_H = KT_H // 2

    with tile.TileContext(nc) as tc:
        with (
            tc.tile_pool(name="resident", bufs=1) as res,
            tc.tile_pool(name="wslab", bufs=2) as wp,
            tc.tile_pool(name="psum", bufs=8, space="PSUM") as pp,
            tc.tile_pool(name="stage", bufs=2) as sp,
            tc.tile_pool(name="bias", bufs=1) as bp,
        ):
            # fp8 pair tiles: [128, 2, 1024]; pair q holds k-tiles 2q, 2q+1
            x8_pairs = [
                res.tile([P, 2, B_LOC], FP8, tag=f"x8{q}", name=f"x8{q}")
                for q in range(NPAIR_X)
            ]
            h8_pairs = [
                res.tile([P, 2, B_LOC], FP8, tag=f"h8{q}", name=f"h8{q}")
                for q in range(NPAIR_H)
            ]
            rh8_pairs = [
                res.tile([P, 2, B_LOC], FP8, tag=f"rh{q}", name=f"rh{q}")
                for q in range(NPAIR_H)
            ]
            # bf16 per-k-tile tiles
            xb_tiles = [
                res.tile([P, B_LOC], BF16, tag=f"xb{j}", name=f"xb{j}")
                for j in range(KT_X)
            ]
            hb_tiles = [
                res.tile([P, B_LOC], BF16, tag=f"hb{j}", name=f"hb{j}")
                for j in range(KT_H)
            ]
            u_tiles = [
                res.tile([P, B_LOC], BF16, tag=f"u{j}", name=f"u{j}")
                for j in range(NT_C)
            ]

            # PE warm-up: the HAM clock gate holds the PE at 1.2 GHz until
            # it has been busy ~3.4us; fill the pre-first-matmul window
            # with dummy matmuls so the PE is un-throttled when real data
            # lands (same trick as the bf16 baseline).
            warm_src = sp.tile(
                [P, M_CHUNK], BF16, tag="warm", name="warm_src", bufs=1
            )
            nc.gpsimd.memset(warm_src[:], 0.0)
            warm_ps = pp.tile([P, M_CHUNK], F32, tag="psum", name="warm_ps")
            for w in range(8):
                nc.tensor.matmul(
                    warm_ps[:],
                    warm_src[:, :P],
                    warm_src[:],
                    start=(w == 0),
                    stop=(w == 7),
                )

            # Startup DMAs in exact consumption order of the first r-gate
            # col-tile pair, interleaved across both HWDGE rings.
            # Graduated chunk sizes (in k-tiles over the 32-long [x; h]
            # sequence); all chunk boundaries are even so DoubleRow pairs
            # never straddle a chunk.
            CHUNKS = [2, 6, 8, 8, 8]
            CB = [0, 2, 8, 16, 24, 32]  # chunk k-tile boundaries
            ws_first = [[None] * len(CHUNKS) for _ in range(2)]  # [t][chunk]
            src_dma = {}  # k-slot -> (engine, dst ap, src ap)
            for j in range(KT_X):
                eng = nc.sync if j % 2 == 0 else nc.scalar
                src_dma[j] = (eng, x8_pairs[j // 2][:, j % 2, :], x8d[j, :, :])
            for j in range(KT_H):
                eng = nc.scalar if j % 2 == 0 else nc.sync
                src_dma[KT_X + j] = (
                    eng, h8_pairs[j // 2][:, j % 2, :], h8d[j, :, :]
                )
            for c, cw in enumerate(CHUNKS):
                for t in range(2):
                    w = wp.tile(
                        [P, cw, P], FP8, tag=f"wr{t}_{c}", name=f"wr{t}_{c}",
                        bufs=1,
                    )
                    (nc.sync if t == 0 else nc.scalar).dma_start(
                        w[:], w_r8[t, :, CB[c]:CB[c + 1], :]
                    )
                    ws_first[t][c] = w
                for j in range(CB[c], CB[c + 1]):
                    eng, dst, src = src_dma[j]
                    if c == 0:
                        # First block via the SWDGE queue, in parallel with
                        # the weight chunks on the two HWDGE rings.
                        eng = nc.gpsimd
                    eng.dma_start(dst, src)

            bg_all = bp.tile([P, NT_G], F32, tag="bg", name="bg_all")
            nc.sync.dma_start(bg_all[:], b_g[:, :])
            bc_all = bp.tile([P, NT_C], F32, tag="bc", name="bc_all")
            nc.scalar.dma_start(bc_all[:], b_c[:, :])

            # bf16 h tiles 0/1 are needed by the t0/t1 r epilogues (~30us
            # in); the rest of hb/xb is interleaved with the steady-state
            # r weight slabs below so the slabs aren't stuck behind 8MB
            # of bf16 input traffic on the HWDGE rings.
            for j in range(4):
                (nc.sync if j % 2 == 0 else nc.scalar).dma_start(
                    hb_tiles[j][:], hbd[j, :, :]
                )

            all_pairs = x8_pairs + h8_pairs  # 16 fp8 pair tiles = 32 k-tiles

            def act_r(t, m, ps):
                """r epilogue: rh8[t] = sigmoid(ps/S_W + b) * h  (fp8)."""
                ms = slice(m * M_CHUNK, (m + 1) * M_CHUNK)
                rt = sp.tile([P, M_CHUNK], BF16, tag="rtmp", name=f"r{t}_{m}")
                nc.scalar.activation(
                    rt[:], ps[:], SIG, bias=bg_all[:, t:t + 1], scale=S_INV
                )
                nc.vector.tensor_mul(
                    rh8_pairs[t // 2][:, t % 2, ms], rt[:], hb_tiles[t][:, ms]
                )

            def act_u(t, m, ps):
                ms = slice(m * M_CHUNK, (m + 1) * M_CHUNK)
                nc.scalar.activation(
                    u_tiles[t - NT_C][:, ms], ps[:], SIG,
                    bias=bg_all[:, t:t + 1], scale=S_INV,
                )

            # ---- Phase R: r gates (cols 0..15), fully fp8 DoubleRow ------
            # First col-tile pair is block-interleaved over the startup
            # chunks (4 psum groups) so the PE has matmuls to run per
            # freshly-arrived chunk instead of stalling on the full set.
            t0_groups = [(0, 0), (0, 1), (1, 0), (1, 1)]
            pss0 = [
                pp.tile([P, M_CHUNK], F32, tag="psum", name=f"psg0_{i}")
                for i in range(4)
            ]
            for c in range(len(CHUNKS)):
                q0, q1 = CB[c] // 2, CB[c + 1] // 2
                for i, (t, m) in enumerate(t0_groups):
                    ms = slice(m * M_CHUNK, (m + 1) * M_CHUNK)
                    for qq in range(q0, q1):
                        jj = qq - q0  # pair index within this chunk's slab
                        nc.tensor.matmul(
                            pss0[i][:],
                            ws_first[t][c][:, 2 * jj:2 * jj + 2, :],
                            all_pairs[qq][:, 0:2, ms],
                            start=(qq == 0),
                            stop=(qq == KT // 2 - 1),
                            perf_mode=DR,
                        )
            for i, (t, m) in enumerate(t0_groups):
                act_r(t, m, pss0[i])

            # Steady-state r cols: one fp8 slab [128, 32, 128] per col-tile,
            # m-interleaved so consecutive matmuls share the stationary
            # weight pair (one 256-col LDWEIGHTS per 2 matmuls).
            for t in range(2, NT_C):
                ws = wp.tile([P, KT, P], FP8, tag="wr", name=f"wr{t}")
                (nc.sync if t % 2 == 0 else nc.scalar).dma_start(
                    ws[:], w_r8[t, :, :, :]
                )
                # pace the bf16 inputs behind the slab they follow:
                # hb[t] lands ~1 col-tile before its epilogue needs it,
                # xb streams in over the back half of the r phase (it is
                # first read in phase U).
                if t < KT_H - 2:
                    (nc.scalar if t % 2 == 0 else nc.sync).dma_start(
                        hb_tiles[t + 2][:], hbd[t + 2, :, :]
                    )
                if t >= 8:
                    j0 = 2 * (t - 8)
                    (nc.scalar if t % 2 == 0 else nc.sync).dma_start(
                        xb_tiles[j0][:], xbd[j0, :, :]
                    )
                    (nc.sync if t % 2 == 0 else nc.scalar).dma_start(
                        xb_tiles[j0 + 1][:], xbd[j0 + 1, :, :]
                    )
                psl = [
                    pp.tile([P, M_CHUNK], F32, tag="psum", name=f"psr{t}_{m}")
                    for m in range(MC)
                ]
                for q in range(KT // 2):
                    for m in range(MC):
                        ms = slice(m * M_CHUNK, (m + 1) * M_CHUNK)
                        nc.tensor.matmul(
                            psl[m][:],
                            ws[:, 2 * q:2 * q + 2, :],
                            all_pairs[q][:, 0:2, ms],
                            start=(q == 0),
                            stop=(q == KT // 2 - 1),
                            perf_mode=DR,
                        )
                for m in range(MC):
                    act_r(t, m, psl[m])

            # ---- Phase U: u gates (cols 16..31), mixed fp8/bf16 ----------
            for u in range(NT_C):
                t = NT_C + u
                w8 = None
                if n8u:
                    w8 = wp.tile([P, n8u, P], FP8, tag="wu8", name=f"wu8_{u}")
                    (nc.sync if u % 2 == 0 else nc.scalar).dma_start(
                        w8[:], w_u8[u, :, :, :]
                    )
                wb = wp.tile([P, nbu, P], BF16, tag="wub", name=f"wub_{u}")
                (nc.scalar if u % 2 == 0 else nc.sync).dma_start(
                    wb[:], w_ub[u, :, :, :]
                )
                psl = [
                    pp.tile([P, M_CHUNK], F32, tag="psum", name=f"psu{t}_{m}")
                    for m in range(MC)
                ]
                n_mm = n8u // 2 + nbu  # total accumulation steps per m
                step = 0
                # fp8 DoubleRow part: x pairs then h pairs
                for q in range(N8_XU // 2):
                    for m in range(MC):
                        ms = slice(m * M_CHUNK, (m + 1) * M_CHUNK)
                        nc.tensor.matmul(
                            psl[m][:],
                            w8[:, 2 * q:2 * q + 2, :],
                            x8_pairs[q][:, 0:2, ms],
                            start=(step == 0),
                            stop=(step == n_mm - 1),
                            perf_mode=DR,
                        )
                    step += 1
                for q in range(N8_HU // 2):
                    off = N8_XU + 2 * q
                    for m in range(MC):
                        ms = slice(m * M_CHUNK, (m + 1) * M_CHUNK)
                        nc.tensor.matmul(
                            psl[m][:],
                            w8[:, off:off + 2, :],
                            h8_pairs[q][:, 0:2, ms],
                            start=(step == 0),
                            stop=(step == n_mm - 1),
                            perf_mode=DR,
                        )
                    step += 1
                # bf16 part: x k-tiles N8_XU..15, then h k-tiles N8_HU..15
                for i, src in enumerate(
                    [xb_tiles[j] for j in range(N8_XU, KT_X)]
                    + [hb_tiles[j] for j in range(N8_HU, KT_H)]
                ):
                    for m in range(MC):
                        ms = slice(m * M_CHUNK, (m + 1) * M_CHUNK)
                        nc.tensor.matmul(
                            psl[m][:],
                            wb[:, i, :],
                            src[:, ms],
                            start=(step == 0),
                            stop=(step == n_mm - 1),
                        )
                    step += 1
                for m in range(MC):
                    act_u(t, m, psl[m])

            # ---- Phase C: candidate + output combine ---------------------
            # psum = (r*h)@Wh3 (fp8 DR) + x@Wi3 (bf16);
            # h_t = u * (h - cand) + cand
            def cand_epilogue(t, m, mw, ps):
                ms = slice(m * mw, (m + 1) * mw)
                cand = sp.tile([P, mw], F32, tag="cand", name=f"c{t}_{m}")
                nc.scalar.activation(
                    cand[:], ps[:], TANH, bias=bc_all[:, t:t + 1], scale=S_INV
                )
                d = sp.tile([P, mw], F32, tag="d", name=f"d{t}_{m}")
                nc.vector.tensor_sub(d[:], hb_tiles[t][:, ms], cand[:])
                d2 = sp.tile([P, mw], F32, tag="d2", name=f"d2{t}_{m}")
                nc.vector.tensor_mul(d2[:], u_tiles[t][:, ms], d[:])
                ht = sp.tile([P, mw], F32, tag="ht", name=f"ht{t}_{m}")
                nc.vector.tensor_add(ht[:], d2[:], cand[:])
                nc.sync.dma_start(out[t, :, ms], ht[:])

            def cand_slabs(t):
                w8 = wp.tile([P, KT_H, P], FP8, tag="wc8", name=f"wc8_{t}")
                (nc.sync if t % 2 == 0 else nc.scalar).dma_start(
                    w8[:], w_c8[t, :, :, :]
                )
                wb = wp.tile([P, KT_X, P], BF16, tag="wcb", name=f"wcb_{t}")
                (nc.scalar if t % 2 == 0 else nc.sync).dma_start(
                    wb[:], w_cb[t, :, :, :]
                )
                return w8, wb

            def cand_accum(w8, wb, ps_of_m, mws):
                """Accumulate candidate GEMM for the m-chunks in ps_of_m
                (dict m -> psum tile), chunk width mws."""
                n_mm = KT_H // 2 + KT_X
                step = 0
                for q in range(KT_H // 2):
                    for m, ps in ps_of_m.items():
                        ms = slice(m * mws, (m + 1) * mws)
                        nc.tensor.matmul(
                            ps[:],
                            w8[:, 2 * q:2 * q + 2, :],
                            rh8_pairs[q][:, 0:2, ms],
                            start=(step == 0),
                            stop=(step == n_mm - 1),
                            perf_mode=DR,
                        )
                    step += 1
                for j in range(KT_X):
                    for m, ps in ps_of_m.items():
                        ms = slice(m * mws, (m + 1) * mws)
                        nc.tensor.matmul(
                            ps[:],
                            wb[:, j, :],
                            xb_tiles[j][:, ms],
                            start=(step == 0),
                            stop=(step == n_mm - 1),
                        )
                    step += 1

            # No taper on the last tile: narrow (256-wide) chunks are
            # LDWEIGHTS-bound (~305ns/matmul regardless of width), so the
            # baseline's taper costs more PE time than the epilogue tail
            # it hides.
            for t in range(NT_C):
                w8, wb = cand_slabs(t)
                psl = {
                    m: pp.tile(
                        [P, M_CHUNK], F32, tag="psum", name=f"psc{t}_{m}"
                    )
                    for m in range(MC)
                }
                cand_accum(w8, wb, psl, M_CHUNK)
                for m in range(MC):
                    cand_epilogue(t, m, M_CHUNK, psl[m])

    nc.compile()
    return nc


def _get_nc():
    global _CACHED_NC
    if _CACHED_NC is None:
        _CACHED_NC = _build()
    return _CACHED_NC


def _ct_blocks(w):
    """[K, N] -> [N/128 col-tiles, K/128 k-tiles, 128p, 128c] blocks."""
    K, N = w.shape
    return np.ascontiguousarray(
        w.reshape(K // P, P, N // P, P).transpose(2, 0, 1, 3)
    )


def _slab(blocks, ct, sel, np_dtype):
    """Pack k-tiles `sel` of col-tile ct into [128p, len(sel), 128c]."""
    a = blocks[ct][sel]  # [nkt, 128p, 128c]
    return np.ascontiguousarray(a.transpose(1, 0, 2)).astype(np_dtype)


def kernel(x_t, h_tm1, input_weight, hidden_state_weight, bias):
    x_t = np.asarray(x_t, dtype=np.float32)
    h_tm1 = np.asarray(h_tm1, dtype=np.float32)
    input_weight = np.asarray(input_weight, dtype=np.float32)
    hidden_state_weight = np.asarray(hidden_state_weight, dtype=np.float32)
    bias = np.asarray(bias, dtype=np.float32)

    u = UNITS
    # Gate weights: [x; h] @ [Wi[:, :2u]; Wh[:, :2u]], pre-scaled by S_W
    w_gate = np.concatenate(
        [input_weight[:, : 2 * u], hidden_state_weight[:, : 2 * u]], axis=0
    ) * np.float32(S_W)  # [4096, 4096]
    w_cand = np.concatenate(
        [input_weight[:, 2 * u:], hidden_state_weight[:, 2 * u:]], axis=0
    ) * np.float32(S_W)  # [4096, 2048]

    bg = _ct_blocks(w_gate)   # [32 ct, 32 kt, 128, 128]
    bc = _ct_blocks(w_cand)   # [16 ct, 32 kt, 128, 128]

    kt_all = list(range(KT))
    sel_u8 = list(range(N8_XU)) + list(range(KT_X, KT_X + N8_HU))
    sel_ub = list(range(N8_XU, KT_X)) + list(range(KT_X + N8_HU, KT))
    sel_c8 = list(range(KT_X, KT))      # rh k-tiles (h rows of w_cand)
    sel_cb = list(range(KT_X))          # x k-tiles

    w_r8_np = np.stack([_slab(bg, t, kt_all, NP_FP8) for t in range(NT_C)])
    w_u8_np = np.stack(
        [_slab(bg, NT_C + t, sel_u8, NP_FP8) for t in range(NT_C)]
    )
    w_ub_np = np.stack(
        [_slab(bg, NT_C + t, sel_ub, NP_BF16) for t in range(NT_C)]
    )
    w_c8_np = np.stack([_slab(bc, t, sel_c8, NP_FP8) for t in range(NT_C)])
    w_cb_np = np.stack([_slab(bc, t, sel_cb, NP_BF16) for t in range(NT_C)])

    b_g_np = np.ascontiguousarray(
        bias[: 2 * u].reshape(NT_G, P).T, dtype=np.float32
    )
    b_c_np = np.ascontiguousarray(
        bias[2 * u:].reshape(NT_C, P).T, dtype=np.float32
    )

    in_maps = []
    for i in range(N_CORES):
        sl = slice(i * B_LOC, (i + 1) * B_LOC)
        xT = x_t[sl].T  # [2048, 1024] fp32
        hT = h_tm1[sl].T
        in_maps.append(
            {
                "x8": np.ascontiguousarray(
                    xT.astype(NP_FP8).reshape(KT_X, P, B_LOC)
                ),
                "h8": np.ascontiguousarray(
                    hT.astype(NP_FP8).reshape(KT_H, P, B_LOC)
                ),
                "xb": np.ascontiguousarray(
                    xT.astype(NP_BF16).reshape(KT_X, P, B_LOC)
                ),
                "hb": np.ascontiguousarray(
                    hT.astype(NP_BF16).reshape(KT_H, P, B_LOC)
                ),
                "w_r8": w_r8_np,
                "w_u8": w_u8_np,
                "w_ub": w_ub_np,
                "w_c8": w_c8_np,
                "w_cb": w_cb_np,
                "b_g": b_g_np,
                "b_c": b_c_np,
            }
        )

    nc = _get_nc()
    res = run_bass_kernel_spmd(
        nc, in_maps, core_ids=list(range(N_CORES)), trace=TRACE
    )
    global LAST_RESULTS
    LAST_RESULTS = res

    h_t = np.empty((BATCH, UNITS), dtype=np.float32)
    for i in range(N_CORES):
        o = np.asarray(res.results[i]["out"], dtype=np.float32)
        h_t[i * B_LOC:(i + 1) * B_LOC] = o.reshape(UNITS, B_LOC).T
    return h_t


# revision 14
# speedup vs baseline: 1.4929x; 1.0139x over previous
"""GRU cell (AnotherGRUCell) on 8 TRN2 NeuronCores.

Strategy: pure data-parallel over batch (8192 rows -> 1024 rows/core),
weights replicated. No collectives.

All on-chip compute is in TRANSPOSED layout (units on the partition axis,
batch on the free axis), as in the bf16 baseline. New here: most of the
GEMM work runs in fp8-e4m3 with perf_mode=DoubleRow (2 k-tiles per PE
instruction), which roughly doubles PE matmul throughput. fp8
quantization noise is steered by a per-unit precision config chosen via
a host-side numpy simulation against the 2e-2 rel-err gate:

  - r gates (cols 0..15):     x@Wr + h@Wr fully fp8 (error is attenuated
                              through r*h -> cand -> (1-u) weighting)
  - u gates (cols 16..31):    first N8_XU/N8_HU k-tiles of x/h in fp8,
                              rest bf16 (u multiplies h directly in the
                              output, so u noise is expensive)
  - cand (r*h)@Wh3:           fully fp8 (attenuated like r)
  - cand x@Wi3:               bf16 (tanh pre-act noise is expensive)

All weights (both dtypes) are pre-scaled by S_W=32 on the host so fp8
sees a ~unit-std distribution, and every PSUM accumulation has one
uniform scale that is divided out for free inside the ScalarE
activation (out = sigmoid/tanh(psum * 1/S_W + bias)).

fp8 x/h/rh operands live in PAIR tiles [128, 2, 1024] so each DoubleRow
matmul gets its required 3D AP [128, 2, free] (pair-dim step % 16 == 0)
while startup DMAs keep per-pair dependency granularity.
"""

import numpy as np
import ml_dtypes

import concourse.bacc as bacc
import concourse.tile as tile
import concourse.mybir as mybir
from concourse.bass_utils import run_bass_kernel_spmd

N_CORES = 8
UNITS = 2048
IN_DIM = 2048
BATCH = 8192
B_LOC = BATCH // N_CORES  # 1024 batch rows per core

P = 128
KT_X = IN_DIM // P           # 16 k-tiles of x
KT_H = UNITS // P            # 16 k-tiles of h
KT = KT_X + KT_H             # 32 contraction k-tiles for [x; h]
NT_G = (2 * UNITS) // P      # 32 gate col-tiles (r: 0..15, u: 16..31)
NT_C = UNITS // P            # 16 candidate col-tiles
M_CHUNK = 512
MC = B_LOC // M_CHUNK        # 2 moving chunks per core

# Precision config: number of leading k-tiles (of 16) computed in fp8
# DoubleRow for the u-gate x/h operands. r gates and (r*h)@Wh3 are fully
# fp8; the candidate x@Wi3 is fully bf16. Must be even (DoubleRow pairs).
# Host-sim predicted rel err (matches HW to ~5 digits): 4 -> 1.571e-2,
# 8 -> 1.811e-2, 10 -> 1.920e-2 against the 2e-2 gate.
N8_XU = 10
N8_HU = 10
S_W = 32.0
S_INV = float(1.0 / S_W)

BF16 = mybir.dt.bfloat16
F32 = mybir.dt.float32
FP8 = mybir.dt.float8e4
NP_BF16 = ml_dtypes.bfloat16
NP_FP8 = ml_dtypes.float8_e4m3  # IEEE-style e4m3, max 240 == TRN FP8_EXP4
DR = mybir.MatmulPerfMode.DoubleRow

_CACHED_NC = None

# test.py sets TRACE=True to capture the NTFF profile (exec_time_ns +
# perfetto trace); the graded path leaves it off. LAST_RESULTS holds the
# BassKernelResults of the most recent run.
TRACE = False
LAST_RESULTS = None


def _build():
    nc = bacc.Bacc("TRN2", target_bir_lowering=False, debug=False)

    # fp8 transposed inputs, one [128, 1024] slice per k-tile
    x8d = nc.dram_tensor("x8", [KT_X, P, B_LOC], FP8, kind="ExternalInput")
    h8d = nc.dram_tensor("h8", [KT_H, P, B_LOC], FP8, kind="ExternalInput")
    # bf16 transposed inputs (u-gate bf16 part, cand x part, epilogues)
    xbd = nc.dram_tensor("xb", [KT_X, P, B_LOC], BF16, kind="ExternalInput")
    hbd = nc.dram_tensor("hb", [KT_H, P, B_LOC], BF16, kind="ExternalInput")
    # weights, pre-scaled by S_W, packed per col-tile as [128p, nkt, 128c]
    n8u = N8_XU + N8_HU
    nbu = KT - n8u
    w_r8 = nc.dram_tensor("w_r8", [NT_C, P, KT, P], FP8, kind="ExternalInput")
    w_u8 = nc.dram_tensor("w_u8", [NT_C, P, n8u, P], FP8, kind="ExternalInput")
    w_ub = nc.dram_tensor("w_ub", [NT_C, P, nbu, P], BF16, kind="ExternalInput")
    w_c8 = nc.dram_tensor("w_c8", [NT_C, P, KT_H, P], FP8, kind="ExternalInput")
    w_cb = nc.dram_tensor("w_cb", [NT_C, P, KT_X, P], BF16, kind="ExternalInput")
    # biases transposed: one [128, n_tiles] tensor per gate set -> 1 DMA each
    b_g = nc.dram_tensor("b_g", [P, NT_G], F32, kind="ExternalInput")
    b_c = nc.dram_tensor("b_c", [P, NT_C], F32, kind="ExternalInput")
    out = nc.dram_tensor("out", [NT_C, P, B_LOC], F32, kind="ExternalOutput")

    SIG = mybir.ActivationFunctionType.Sigmoid
    TANH = mybir.ActivationFunctionType.Tanh

    NPAIR_X = KT_X // 2
    NPAIR_H = KT_H // 2

    with tile.TileContext(nc) as tc:
        with (
            tc.tile_pool(name="resident", bufs=1) as res,
            tc.tile_pool(name="wslab", bufs=2) as wp,
            tc.tile_pool(name="psum", bufs=8, space="PSUM") as pp,
            tc.tile_pool(name="stage", bufs=2) as sp,
            tc.tile_pool(name="bias", bufs=1) as bp,
        ):
            # fp8 pair tiles: [128, 2, 1024]; pair q holds k-tiles 2q, 2q+1
            x8_pairs = [
                res.tile([P, 2, B_LOC], FP8, tag=f"x8{q}", name=f"x8{q}")
                for q in range(NPAIR_X)
            ]
            h8_pairs = [
                res.tile([P, 2, B_LOC], FP8, tag=f"h8{q}", name=f"h8{q}")
                for q in range(NPAIR_H)
            ]
            rh8_pairs = [
                res.tile([P, 2, B_LOC], FP8, tag=f"rh{q}", name=f"rh{q}")
                for q in range(NPAIR_H)
            ]
            # bf16 per-k-tile tiles
            xb_tiles = [
                res.tile([P, B_LOC], BF16, tag=f"xb{j}", name=f"xb{j}")
                for j in range(KT_X)
            ]
            hb_tiles = [
                res.tile([P, B_LOC], BF16, tag=f"hb{j}", name=f"hb{j}")
                for j in range(KT_H)
            ]
            u_tiles = [
                res.tile([P, B_LOC], BF16, tag=f"u{j}", name=f"u{j}")
                for j in range(NT_C)
            ]

            # PE warm-up: the HAM clock gate holds the PE at 1.2 GHz until
            # it has been busy ~3.4us; fill the pre-first-matmul window
            # with dummy matmuls so the PE is un-throttled when real data
            # lands (same trick as the bf16 baseline).
            warm_src = sp.tile(
                [P, M_CHUNK], BF16, tag="warm", name="warm_src", bufs=1
            )
            nc.gpsimd.memset(warm_src[:], 0.0)
            warm_ps = pp.tile([P, M_CHUNK], F32, tag="psum", name="warm_ps")
            for w in range(8):
                nc.tensor.matmul(
                    warm_ps[:],
                    warm_src[:, :P],
                    warm_src[:],
                    start=(w == 0),
                    stop=(w == 7),
                )

            # Startup DMAs in exact consumption order of the first r-gate
            # col-tile pair, interleaved across both HWDGE rings.
            # Graduated chunk sizes (in k-tiles over the 32-long [x; h]
            # sequence); all chunk boundaries are even so DoubleRow pairs
            # never straddle a chunk.
            CHUNKS = [2, 6, 8, 8, 8]
            CB = [0, 2, 8, 16, 24, 32]  # chunk k-tile boundaries
            ws_first = [[None] * len(CHUNKS) for _ in range(2)]  # [t][chunk]
            src_dma = {}  # k-slot -> (engine, dst ap, src ap)
            for j in range(KT_X):
                eng = nc.sync if j % 2 == 0 else nc.scalar
                src_dma[j] = (eng, x8_pairs[j // 2][:, j % 2, :], x8d[j, :, :])
            for j in range(KT_H):
                eng = nc.scalar if j % 2 == 0 else nc.sync
                src_dma[KT_X + j] = (
                    eng, h8_pairs[j // 2][:, j % 2, :], h8d[j, :, :]
                )
            pre_ws = {}
            for c, cw in enumerate(CHUNKS):
                if c == len(CHUNKS) - 1:
                    # Sneak the first two steady-state r slabs in ahead of
                    # the last startup chunk: t=2's slab gates the PE at
                    # ~28us and must not sit behind the bf16 input stream.
                    for t in (2, 3):
                        ws = wp.tile([P, KT, P], FP8, tag="wr", name=f"wr{t}", bufs=3)
                        (nc.sync if t % 2 == 0 else nc.scalar).dma_start(
                            ws[:], w_r8[t, :, :, :]
                        )
                        pre_ws[t] = ws
                for t in range(2):
                    w = wp.tile(
                        [P, cw, P], FP8, tag=f"wr{t}_{c}", name=f"wr{t}_{c}",
                        bufs=1,
                    )
                    (nc.sync if t == 0 else nc.scalar).dma_start(
                        w[:], w_r8[t, :, CB[c]:CB[c + 1], :]
                    )
                    ws_first[t][c] = w
                for j in range(CB[c], CB[c + 1]):
                    eng, dst, src = src_dma[j]
                    if c == 0:
                        # First block via the SWDGE queue, in parallel with
                        # the weight chunks on the two HWDGE rings.
                        eng = nc.gpsimd
                    eng.dma_start(dst, src)

            # Biases + the early bf16 h tiles (needed by the first r
            # epilogues ~30us in) go on the SWDGE queue: the two HWDGE
            # rings deliver ~100GB/s each and are fully booked with the
            # startup x8/h8/weight traffic that gates the PE.
            bg_all = bp.tile([P, NT_G], F32, tag="bg", name="bg_all")
            nc.gpsimd.dma_start(bg_all[:], b_g[:, :])
            bc_all = bp.tile([P, NT_C], F32, tag="bc", name="bc_all")
            nc.gpsimd.dma_start(bc_all[:], b_c[:, :])
            for j in range(4):
                nc.gpsimd.dma_start(hb_tiles[j][:], hbd[j, :, :])

            all_pairs = x8_pairs + h8_pairs  # 16 fp8 pair tiles = 32 k-tiles

            def act_r(t, m, ps):
                """r epilogue: rh8[t] = sigmoid(ps/S_W + b) * h  (fp8)."""
                ms = slice(m * M_CHUNK, (m + 1) * M_CHUNK)
                rt = sp.tile([P, M_CHUNK], BF16, tag="rtmp", name=f"r{t}_{m}")
                nc.scalar.activation(
                    rt[:], ps[:], SIG, bias=bg_all[:, t:t + 1], scale=S_INV
                )
                nc.vector.tensor_mul(
                    rh8_pairs[t // 2][:, t % 2, ms], rt[:], hb_tiles[t][:, ms]
                )

            def act_u(t, m, ps):
                ms = slice(m * M_CHUNK, (m + 1) * M_CHUNK)
                nc.scalar.activation(
                    u_tiles[t - NT_C][:, ms], ps[:], SIG,
                    bias=bg_all[:, t:t + 1], scale=S_INV,
                )

            # ---- Phase R: r gates (cols 0..15), fully fp8 DoubleRow ------
            # First col-tile pair is block-interleaved over the startup
            # chunks (4 psum groups) so the PE has matmuls to run per
            # freshly-arrived chunk instead of stalling on the full set.
            t0_groups = [(0, 0), (0, 1), (1, 0), (1, 1)]
            pss0 = [
                pp.tile([P, M_CHUNK], F32, tag="psum", name=f"psg0_{i}")
                for i in range(4)
            ]
            for c in range(len(CHUNKS)):
                q0, q1 = CB[c] // 2, CB[c + 1] // 2
                for i, (t, m) in enumerate(t0_groups):
                    ms = slice(m * M_CHUNK, (m + 1) * M_CHUNK)
                    for qq in range(q0, q1):
                        jj = qq - q0  # pair index within this chunk's slab
                        nc.tensor.matmul(
                            pss0[i][:],
                            ws_first[t][c][:, 2 * jj:2 * jj + 2, :],
                            all_pairs[qq][:, 0:2, ms],
                            start=(qq == 0),
                            stop=(qq == KT // 2 - 1),
                            perf_mode=DR,
                        )
            for i, (t, m) in enumerate(t0_groups):
                act_r(t, m, pss0[i])

            # Steady-state r cols: one fp8 slab [128, 32, 128] per col-tile,
            # m-interleaved so consecutive matmuls share the stationary
            # weight pair (one 256-col LDWEIGHTS per 2 matmuls).
            for t in range(2, NT_C):
                if t in pre_ws:
                    ws = pre_ws[t]
                else:
                    ws = wp.tile([P, KT, P], FP8, tag="wr", name=f"wr{t}", bufs=3)
                    (nc.sync if t % 2 == 0 else nc.scalar).dma_start(
                        ws[:], w_r8[t, :, :, :]
                    )
                # pace the bf16 inputs behind the slab they follow:
                # hb[t] lands ~1 col-tile before its epilogue needs it,
                # xb streams in over the back half of the r phase (it is
                # first read in phase U).
                if t < KT_H - 2:
                    (nc.scalar if t % 2 == 0 else nc.sync).dma_start(
                        hb_tiles[t + 2][:], hbd[t + 2, :, :]
                    )
                if t >= 8:
                    j0 = 2 * (t - 8)
                    (nc.scalar if t % 2 == 0 else nc.sync).dma_start(
                        xb_tiles[j0][:], xbd[j0, :, :]
                    )
                    (nc.sync if t % 2 == 0 else nc.scalar).dma_start(
                        xb_tiles[j0 + 1][:], xbd[j0 + 1, :, :]
                    )
                psl = [
                    pp.tile([P, M_CHUNK], F32, tag="psum", name=f"psr{t}_{m}")
                    for m in range(MC)
                ]
                for q in range(KT // 2):
                    for m in range(MC):
                        ms = slice(m * M_CHUNK, (m + 1) * M_CHUNK)
                        nc.tensor.matmul(
                            psl[m][:],
                            ws[:, 2 * q:2 * q + 2, :],
                            all_pairs[q][:, 0:2, ms],
                            start=(q == 0),
                            stop=(q == KT // 2 - 1),
                            perf_mode=DR,
                        )
                for m in range(MC):
                    act_r(t, m, psl[m])

            # ---- Phase U: u gates (cols 16..31), mixed fp8/bf16 ----------
            for u in range(NT_C):
                t = NT_C + u
                w8 = None
                if n8u:
                    w8 = wp.tile([P, n8u, P], FP8, tag="wu8", name=f"wu8_{u}")
                    (nc.sync if u % 2 == 0 else nc.scalar).dma_start(
                        w8[:], w_u8[u, :, :, :]
                    )
                wb = wp.tile([P, nbu, P], BF16, tag="wub", name=f"wub_{u}")
                (nc.scalar if u % 2 == 0 else nc.sync).dma_start(
                    wb[:], w_ub[u, :, :, :]
                )
                psl = [
                    pp.tile([P, M_CHUNK], F32, tag="psum", name=f"psu{t}_{m}")
                    for m in range(MC)
                ]
                n_mm = n8u // 2 + nbu  # total accumulation steps per m
                step = 0
                # fp8 DoubleRow part: x pairs then h pairs
                for q in range(N8_XU // 2):
                    for m in range(MC):
                        ms = slice(m * M_CHUNK, (m + 1) * M_CHUNK)
                        nc.tensor.matmul(
                            psl[m][:],
                            w8[:, 2 * q:2 * q + 2, :],
                            x8_pairs[q][:, 0:2, ms],
                            start=(step == 0),
                            stop=(step == n_mm - 1),
                            perf_mode=DR,
                        )
                    step += 1
                for q in range(N8_HU // 2):
                    off = N8_XU + 2 * q
                    for m in range(MC):
                        ms = slice(m * M_CHUNK, (m + 1) * M_CHUNK)
                        nc.tensor.matmul(
                            psl[m][:],
                            w8[:, off:off + 2, :],
                            h8_pairs[q][:, 0:2, ms],
                            start=(step == 0),
                            stop=(step == n_mm - 1),
                            perf_mode=DR,
                        )
                    step += 1
                # bf16 part: x k-tiles N8_XU..15, then h k-tiles N8_HU..15
                for i, src in enumerate(
                    [xb_tiles[j] for j in range(N8_XU, KT_X)]
                    + [hb_tiles[j] for j in range(N8_HU, KT_H)]
                ):
                    for m in range(MC):
                        ms = slice(m * M_CHUNK, (m + 1) * M_CHUNK)
                        nc.tensor.matmul(
                            psl[m][:],
                            wb[:, i, :],
                            src[:, ms],
                            start=(step == 0),
                            stop=(step == n_mm - 1),
                        )
                    step += 1
                for m in range(MC):
                    act_u(t, m, psl[m])

            # ---- Phase C: candidate + output combine ---------------------
            # psum = (r*h)@Wh3 (fp8 DR) + x@Wi3 (bf16);
            # h_t = u * (h - cand) + cand
            def cand_epilogue(t, m, mw, ps):
                ms = slice(m * mw, (m + 1) * mw)
                cand = sp.tile([P, mw], F32, tag="cand", name=f"c{t}_{m}")
                nc.scalar.activation(
                    cand[:], ps[:], TANH, bias=bc_all[:, t:t + 1], scale=S_INV
                )
                d = sp.tile([P, mw], F32, tag="d", name=f"d{t}_{m}")
                nc.vector.tensor_sub(d[:], hb_tiles[t][:, ms], cand[:])
                d2 = sp.tile([P, mw], F32, tag="d2", name=f"d2{t}_{m}")
                nc.vector.tensor_mul(d2[:], u_tiles[t][:, ms], d[:])
                ht = sp.tile([P, mw], F32, tag="ht", name=f"ht{t}_{m}")
                nc.vector.tensor_add(ht[:], d2[:], cand[:])
                # SWDGE: keeps the HWDGE rings clear for the cand
                # weight slabs (out DMAs ahead of the last slabs in the
                # sync queue cost ~13us of PE idle at the tail).
                nc.gpsimd.dma_start(out[t, :, ms], ht[:])

            def cand_slabs(t):
                w8 = wp.tile([P, KT_H, P], FP8, tag="wc8", name=f"wc8_{t}")
                (nc.sync if t % 2 == 0 else nc.scalar).dma_start(
                    w8[:], w_c8[t, :, :, :]
                )
                wb = wp.tile([P, KT_X, P], BF16, tag="wcb", name=f"wcb_{t}")
                (nc.scalar if t % 2 == 0 else nc.sync).dma_start(
                    wb[:], w_cb[t, :, :, :]
                )
                return w8, wb

            def cand_accum(w8, wb, ps_of_m, mws):
                """Accumulate candidate GEMM for the m-chunks in ps_of_m
                (dict m -> psum tile), chunk width mws."""
                n_mm = KT_H // 2 + KT_X
                step = 0
                for q in range(KT_H // 2):
                    for m, ps in ps_of_m.items():
                        ms = slice(m * mws, (m + 1) * mws)
                        nc.tensor.matmul(
                            ps[:],
                            w8[:, 2 * q:2 * q + 2, :],
                            rh8_pairs[q][:, 0:2, ms],
                            start=(step == 0),
                            stop=(step == n_mm - 1),
                            perf_mode=DR,
                        )
                    step += 1
                for j in range(KT_X):
                    for m, ps in ps_of_m.items():
                        ms = slice(m * mws, (m + 1) * mws)
                        nc.tensor.matmul(
                            ps[:],
                            wb[:, j, :],
                            xb_tiles[j][:, ms],
                            start=(step == 0),
                            stop=(step == n_mm - 1),
                        )
                    step += 1

            # No taper on the last tile: narrow (256-wide) chunks are
            # LDWEIGHTS-bound (~305ns/matmul regardless of width), so the
            # baseline's taper costs more PE time than the epilogue tail
            # it hides.
            for t in range(NT_C):
                w8, wb = cand_slabs(t)
                psl = {
                    m: pp.tile(
                        [P, M_CHUNK], F32, tag="psum", name=f"psc{t}_{m}"
                    )
                    for m in range(MC)
                }
                cand_accum(w8, wb, psl, M_CHUNK)
                for m in range(MC):
                    cand_epilogue(t, m, M_CHUNK, psl[m])

    nc.compile()
    return nc


def _get_nc():
    global _CACHED_NC
    if _CACHED_NC is None:
        _CACHED_NC = _build()
    return _CACHED_NC


def _ct_blocks(w):
    """[K, N] -> [N/128 col-tiles, K/128 k-tiles, 128p, 128c] blocks."""
    K, N = w.shape
    return np.ascontiguousarray(
        w.reshape(K // P, P, N // P, P).transpose(2, 0, 1, 3)
    )


def _slab(blocks, ct, sel, np_dtype):
    """Pack k-tiles `sel` of col-tile ct into [128p, len(sel), 128c]."""
    a = blocks[ct][sel]  # [nkt, 128p, 128c]
    return np.ascontiguousarray(a.transpose(1, 0, 2)).astype(np_dtype)


def kernel(x_t, h_tm1, input_weight, hidden_state_weight, bias):
    x_t = np.asarray(x_t, dtype=np.float32)
    h_tm1 = np.asarray(h_tm1, dtype=np.float32)
    input_weight = np.asarray(input_weight, dtype=np.float32)
    hidden_state_weight = np.asarray(hidden_state_weight, dtype=np.float32)
    bias = np.asarray(bias, dtype=np.float32)

    u = UNITS
    # Gate weights: [x; h] @ [Wi[:, :2u]; Wh[:, :2u]], pre-scaled by S_W
    w_gate = np.concatenate(
        [input_weight[:, : 2 * u], hidden_state_weight[:, : 2 * u]], axis=0
    ) * np.float32(S_W)  # [4096, 4096]
    w_cand = np.concatenate(
        [input_weight[:, 2 * u:], hidden_state_weight[:, 2 * u:]], axis=0
    ) * np.float32(S_W)  # [4096, 2048]

    bg = _ct_blocks(w_gate)   # [32 ct, 32 kt, 128, 128]
    bc = _ct_blocks(w_cand)   # [16 ct, 32 kt, 128, 128]

    kt_all = list(range(KT))
    sel_u8 = list(range(N8_XU)) + list(range(KT_X, KT_X + N8_HU))
    sel_ub = list(range(N8_XU, KT_X)) + list(range(KT_X + N8_HU, KT))
    sel_c8 = list(range(KT_X, KT))      # rh k-tiles (h rows of w_cand)
    sel_cb = list(range(KT_X))          # x k-tiles

    w_r8_np = np.stack([_slab(bg, t, kt_all, NP_FP8) for t in range(NT_C)])
    w_u8_np = np.stack(
        [_slab(bg, NT_C + t, sel_u8, NP_FP8) for t in range(NT_C)]
    )
    w_ub_np = np.stack(
        [_slab(bg, NT_C + t, sel_ub, NP_BF16) for t in range(NT_C)]
    )
    w_c8_np = np.stack([_slab(bc, t, sel_c8, NP_FP8) for t in range(NT_C)])
    w_cb_np = np.stack([_slab(bc, t, sel_cb, NP_BF16) for t in range(NT_C)])

    b_g_np = np.ascontiguousarray(
        bias[: 2 * u].reshape(NT_G, P).T, dtype=np.float32
    )
    b_c_np = np.ascontiguousarray(
        bias[2 * u:].reshape(NT_C, P).T, dtype=np.float32
    )

    in_maps = []
    for i in range(N_CORES):
        sl = slice(i * B_LOC, (i + 1) * B_LOC)
        xT = x_t[sl].T  # [2048, 1024] fp32
        hT = h_tm1[sl].T
        in_maps.append(
            {
                "x8": np.ascontiguousarray(
                    xT.astype(NP_FP8).reshape(KT_X, P, B_LOC)
                ),
                "h8": np.ascontiguousarray(
                    hT.astype(NP_FP8).reshape(KT_H, P, B_LOC)
                ),
                "xb": np.ascontiguousarray(
                    xT.astype(NP_BF16).reshape(KT_X, P, B_LOC)
                ),
                "hb": np.ascontiguousarray(
                    hT.astype(NP_BF16).reshape(KT_H, P, B_LOC)
                ),
                "w_r8": w_r8_np,
                "w_u8": w_u8_np,
                "w_ub": w_ub_np,
                "w_c8": w_c8_np,
                "w_cb": w_cb_np,
                "b_g": b_g_np,
                "b_c": b_c_np,
            }
        )

    nc = _get_nc()
    res = run_bass_kernel_spmd(
        nc, in_maps, core_ids=list(range(N_CORES)), trace=TRACE
    )
    global LAST_RESULTS
    LAST_RESULTS = res

    h_t = np.empty((BATCH, UNITS), dtype=np.float32)
    for i in range(N_CORES):
        o = np.asarray(res.results[i]["out"], dtype=np.float32)
        h_t[i * B_LOC:(i + 1) * B_LOC] = o.reshape(UNITS, B_LOC).T
    return h_t


# revision 17
# speedup vs baseline: 1.5147x; 1.0146x over previous
"""GRU cell (AnotherGRUCell) on 8 TRN2 NeuronCores.

Strategy: pure data-parallel over batch (8192 rows -> 1024 rows/core),
weights replicated. No collectives.

All on-chip compute is in TRANSPOSED layout (units on the partition axis,
batch on the free axis), as in the bf16 baseline. New here: most of the
GEMM work runs in fp8-e4m3 with perf_mode=DoubleRow (2 k-tiles per PE
instruction), which roughly doubles PE matmul throughput. fp8
quantization noise is steered by a per-unit precision config chosen via
a host-side numpy simulation against the 2e-2 rel-err gate:

  - r gates (cols 0..15):     x@Wr + h@Wr fully fp8 (error is attenuated
                              through r*h -> cand -> (1-u) weighting)
  - u gates (cols 16..31):    first N8_XU/N8_HU k-tiles of x/h in fp8,
                              rest bf16 (u multiplies h directly in the
                              output, so u noise is expensive)
  - cand (r*h)@Wh3:           fully fp8 (attenuated like r)
  - cand x@Wi3:               bf16 (tanh pre-act noise is expensive)

All weights (both dtypes) are pre-scaled by S_W=32 on the host so fp8
sees a ~unit-std distribution, and every PSUM accumulation has one
uniform scale that is divided out for free inside the ScalarE
activation (out = sigmoid/tanh(psum * 1/S_W + bias)).

fp8 x/h/rh operands live in PAIR tiles [128, 2, 1024] so each DoubleRow
matmul gets its required 3D AP [128, 2, free] (pair-dim step % 16 == 0)
while startup DMAs keep per-pair dependency granularity.
"""

import numpy as np
import ml_dtypes

import concourse.bacc as bacc
import concourse.tile as tile
import concourse.mybir as mybir
from concourse.bass_utils import run_bass_kernel_spmd

N_CORES = 8
UNITS = 2048
IN_DIM = 2048
BATCH = 8192
B_LOC = BATCH // N_CORES  # 1024 batch rows per core

P = 128
KT_X = IN_DIM // P           # 16 k-tiles of x
KT_H = UNITS // P            # 16 k-tiles of h
KT = KT_X + KT_H             # 32 contraction k-tiles for [x; h]
NT_G = (2 * UNITS) // P      # 32 gate col-tiles (r: 0..15, u: 16..31)
NT_C = UNITS // P            # 16 candidate col-tiles
M_CHUNK = 512
MC = B_LOC // M_CHUNK        # 2 moving chunks per core

# Precision config: number of leading k-tiles (of 16) computed in fp8
# DoubleRow for the u-gate x/h operands. r gates and (r*h)@Wh3 are fully
# fp8; the candidate x@Wi3 is fully bf16. Must be even (DoubleRow pairs).
# Host-sim predicted rel err (matches HW to ~5 digits): 4 -> 1.571e-2,
# 8 -> 1.811e-2, 10 -> 1.920e-2 against the 2e-2 gate.
N8_XU = 10
N8_HU = 10
S_W = 32.0
S_INV = float(1.0 / S_W)

BF16 = mybir.dt.bfloat16
F32 = mybir.dt.float32
FP8 = mybir.dt.float8e4
NP_BF16 = ml_dtypes.bfloat16
NP_FP8 = ml_dtypes.float8_e4m3  # IEEE-style e4m3, max 240 == TRN FP8_EXP4
DR = mybir.MatmulPerfMode.DoubleRow

_CACHED_NC = None

# test.py sets TRACE=True to capture the NTFF profile (exec_time_ns +
# perfetto trace); the graded path leaves it off. LAST_RESULTS holds the
# BassKernelResults of the most recent run.
TRACE = False
LAST_RESULTS = None


def _build():
    nc = bacc.Bacc("TRN2", target_bir_lowering=False, debug=False)

    # fp8 transposed inputs, one [128, 1024] slice per k-tile
    x8d = nc.dram_tensor("x8", [KT_X, P, B_LOC], FP8, kind="ExternalInput")
    h8d = nc.dram_tensor("h8", [KT_H, P, B_LOC], FP8, kind="ExternalInput")
    # bf16 transposed inputs (u-gate bf16 part, cand x part, epilogues)
    xbd = nc.dram_tensor("xb", [KT_X, P, B_LOC], BF16, kind="ExternalInput")
    hbd = nc.dram_tensor("hb", [KT_H, P, B_LOC], BF16, kind="ExternalInput")
    # weights, pre-scaled by S_W, packed per col-tile as [128p, nkt, 128c]
    n8u = N8_XU + N8_HU
    nbu = KT - n8u
    w_r8 = nc.dram_tensor("w_r8", [NT_C, P, KT, P], FP8, kind="ExternalInput")
    w_u8 = nc.dram_tensor("w_u8", [NT_C, P, n8u, P], FP8, kind="ExternalInput")
    w_ub = nc.dram_tensor("w_ub", [NT_C, P, nbu, P], BF16, kind="ExternalInput")
    w_c8 = nc.dram_tensor("w_c8", [NT_C, P, KT_H, P], FP8, kind="ExternalInput")
    w_cb = nc.dram_tensor("w_cb", [NT_C, P, KT_X, P], BF16, kind="ExternalInput")
    # biases transposed: one [128, n_tiles] tensor per gate set -> 1 DMA each
    b_g = nc.dram_tensor("b_g", [P, NT_G], F32, kind="ExternalInput")
    b_c = nc.dram_tensor("b_c", [P, NT_C], F32, kind="ExternalInput")
    out = nc.dram_tensor("out", [NT_C, P, B_LOC], F32, kind="ExternalOutput")

    SIG = mybir.ActivationFunctionType.Sigmoid
    TANH = mybir.ActivationFunctionType.Tanh

    NPAIR_X = KT_X // 2
    NPAIR_H = KT_H // 2

    with tile.TileContext(nc) as tc:
        with (
            tc.tile_pool(name="resident", bufs=1) as res,
            tc.tile_pool(name="wslab", bufs=2) as wp,
            tc.tile_pool(name="psum", bufs=8, space="PSUM") as pp,
            tc.tile_pool(name="stage", bufs=2) as sp,
            tc.tile_pool(name="bias", bufs=1) as bp,
        ):
            # fp8 pair tiles: [128, 2, 1024]; pair q holds k-tiles 2q, 2q+1
            x8_pairs = [
                res.tile([P, 2, B_LOC], FP8, tag=f"x8{q}", name=f"x8{q}")
                for q in range(NPAIR_X)
            ]
            h8_pairs = [
                res.tile([P, 2, B_LOC], FP8, tag=f"h8{q}", name=f"h8{q}")
                for q in range(NPAIR_H)
            ]
            rh8_pairs = [
                res.tile([P, 2, B_LOC], FP8, tag=f"rh{q}", name=f"rh{q}")
                for q in range(NPAIR_H)
            ]
            # bf16 per-k-tile tiles
            xb_tiles = [
                res.tile([P, B_LOC], BF16, tag=f"xb{j}", name=f"xb{j}")
                for j in range(KT_X)
            ]
            hb_tiles = [
                res.tile([P, B_LOC], BF16, tag=f"hb{j}", name=f"hb{j}")
                for j in range(KT_H)
            ]
            u_tiles = [
                res.tile([P, B_LOC], BF16, tag=f"u{j}", name=f"u{j}")
                for j in range(NT_C)
            ]

            # PE warm-up: the HAM clock gate holds the PE at 1.2 GHz until
            # it has been busy ~3.4us; fill the pre-first-matmul window
            # with dummy matmuls so the PE is un-throttled when real data
            # lands (same trick as the bf16 baseline).
            warm_src = sp.tile(
                [P, M_CHUNK], BF16, tag="warm", name="warm_src", bufs=1
            )
            nc.gpsimd.memset(warm_src[:], 0.0)
            warm_ps = pp.tile([P, M_CHUNK], F32, tag="psum", name="warm_ps")
            for w in range(8):
                nc.tensor.matmul(
                    warm_ps[:],
                    warm_src[:, :P],
                    warm_src[:],
                    start=(w == 0),
                    stop=(w == 7),
                )

            # Startup DMAs in exact consumption order of the first r-gate
            # col-tile pair, interleaved across both HWDGE rings.
            # Graduated chunk sizes (in k-tiles over the 32-long [x; h]
            # sequence); all chunk boundaries are even so DoubleRow pairs
            # never straddle a chunk.
            CHUNKS = [2, 6, 8, 8, 8]
            CB = [0, 2, 8, 16, 24, 32]  # chunk k-tile boundaries
            ws_first = [[None] * len(CHUNKS) for _ in range(2)]  # [t][chunk]
            src_dma = {}  # k-slot -> (engine, dst ap, src ap)
            for j in range(KT_X):
                eng = nc.sync if j % 2 == 0 else nc.scalar
                src_dma[j] = (eng, x8_pairs[j // 2][:, j % 2, :], x8d[j, :, :])
            for j in range(KT_H):
                eng = nc.scalar if j % 2 == 0 else nc.sync
                src_dma[KT_X + j] = (
                    eng, h8_pairs[j // 2][:, j % 2, :], h8d[j, :, :]
                )
            pre_ws = {}
            for c, cw in enumerate(CHUNKS):
                if c == 0:
                    # The very first matmul's operands go FIRST in each
                    # ring queue: x8 pair 0 then the first weight chunk.
                    for j in range(CB[0], CB[1]):
                        eng, dst, src = src_dma[j]
                        eng.dma_start(dst, src)
                if c == len(CHUNKS) - 1:
                    # Sneak the first steady-state r slabs in ahead of
                    # the last startup chunk: t=2's slab gates the PE at
                    # ~28us and must not sit behind the bf16 input stream.
                    for t in (2, 3, 4):
                        ws = wp.tile([P, KT, P], FP8, tag="wr", name=f"wr{t}", bufs=3)
                        (nc.sync if t % 2 == 0 else nc.scalar).dma_start(
                            ws[:], w_r8[t, :, :, :]
                        )
                        pre_ws[t] = ws
                for t in range(2):
                    w = wp.tile(
                        [P, cw, P], FP8, tag=f"wr{t}_{c}", name=f"wr{t}_{c}",
                        bufs=1,
                    )
                    (nc.sync if t == 0 else nc.scalar).dma_start(
                        w[:], w_r8[t, :, CB[c]:CB[c + 1], :]
                    )
                    ws_first[t][c] = w
                if c > 0:
                    for j in range(CB[c], CB[c + 1]):
                        eng, dst, src = src_dma[j]
                        eng.dma_start(dst, src)

            # Biases + the early bf16 h tiles (needed by the first r
            # epilogues ~30us in) go on the SWDGE queue: the two HWDGE
            # rings deliver ~100GB/s each and are fully booked with the
            # startup x8/h8/weight traffic that gates the PE.
            bg_all = bp.tile([P, NT_G], F32, tag="bg", name="bg_all")
            nc.gpsimd.dma_start(bg_all[:], b_g[:, :])
            bc_all = bp.tile([P, NT_C], F32, tag="bc", name="bc_all")
            nc.gpsimd.dma_start(bc_all[:], b_c[:, :])
            for j in range(4):
                nc.gpsimd.dma_start(hb_tiles[j][:], hbd[j, :, :])

            all_pairs = x8_pairs + h8_pairs  # 16 fp8 pair tiles = 32 k-tiles

            def act_r(t, m, ps):
                """r epilogue: rh8[t] = sigmoid(ps/S_W + b) * h  (fp8)."""
                ms = slice(m * M_CHUNK, (m + 1) * M_CHUNK)
                rt = sp.tile([P, M_CHUNK], BF16, tag="rtmp", name=f"r{t}_{m}")
                nc.scalar.activation(
                    rt[:], ps[:], SIG, bias=bg_all[:, t:t + 1], scale=S_INV
                )
                nc.vector.tensor_mul(
                    rh8_pairs[t // 2][:, t % 2, ms], rt[:], hb_tiles[t][:, ms]
                )

            def act_u(t, m, ps):
                ms = slice(m * M_CHUNK, (m + 1) * M_CHUNK)
                nc.scalar.activation(
                    u_tiles[t - NT_C][:, ms], ps[:], SIG,
                    bias=bg_all[:, t:t + 1], scale=S_INV,
                )

            # ---- Phase R: r gates (cols 0..15), fully fp8 DoubleRow ------
            # First col-tile pair is block-interleaved over the startup
            # chunks (4 psum groups) so the PE has matmuls to run per
            # freshly-arrived chunk instead of stalling on the full set.
            t0_groups = [(0, 0), (0, 1), (1, 0), (1, 1)]
            pss0 = [
                pp.tile([P, M_CHUNK], F32, tag="psum", name=f"psg0_{i}")
                for i in range(4)
            ]
            for c in range(len(CHUNKS)):
                q0, q1 = CB[c] // 2, CB[c + 1] // 2
                for i, (t, m) in enumerate(t0_groups):
                    ms = slice(m * M_CHUNK, (m + 1) * M_CHUNK)
                    for qq in range(q0, q1):
                        jj = qq - q0  # pair index within this chunk's slab
                        nc.tensor.matmul(
                            pss0[i][:],
                            ws_first[t][c][:, 2 * jj:2 * jj + 2, :],
                            all_pairs[qq][:, 0:2, ms],
                            start=(qq == 0),
                            stop=(qq == KT // 2 - 1),
                            perf_mode=DR,
                        )
            for i, (t, m) in enumerate(t0_groups):
                act_r(t, m, pss0[i])

            # Steady-state r cols: one fp8 slab [128, 32, 128] per col-tile,
            # m-interleaved so consecutive matmuls share the stationary
            # weight pair (one 256-col LDWEIGHTS per 2 matmuls).
            for t in range(2, NT_C):
                if t in pre_ws:
                    ws = pre_ws[t]
                else:
                    ws = wp.tile([P, KT, P], FP8, tag="wr", name=f"wr{t}", bufs=3)
                    (nc.sync if t % 2 == 0 else nc.scalar).dma_start(
                        ws[:], w_r8[t, :, :, :]
                    )
                # pace the bf16 inputs behind the slab they follow:
                # hb[t] lands ~1 col-tile before its epilogue needs it,
                # xb streams in over the back half of the r phase (it is
                # first read in phase U).
                if t < KT_H - 2:
                    (nc.scalar if t % 2 == 0 else nc.sync).dma_start(
                        hb_tiles[t + 2][:], hbd[t + 2, :, :]
                    )
                if t >= 8:
                    j0 = 2 * (t - 8)
                    (nc.scalar if t % 2 == 0 else nc.sync).dma_start(
                        xb_tiles[j0][:], xbd[j0, :, :]
                    )
                    (nc.sync if t % 2 == 0 else nc.scalar).dma_start(
                        xb_tiles[j0 + 1][:], xbd[j0 + 1, :, :]
                    )
                psl = [
                    pp.tile([P, M_CHUNK], F32, tag="psum", name=f"psr{t}_{m}")
                    for m in range(MC)
                ]
                for q in range(KT // 2):
                    for m in range(MC):
                        ms = slice(m * M_CHUNK, (m + 1) * M_CHUNK)
                        nc.tensor.matmul(
                            psl[m][:],
                            ws[:, 2 * q:2 * q + 2, :],
                            all_pairs[q][:, 0:2, ms],
                            start=(q == 0),
                            stop=(q == KT // 2 - 1),
                            perf_mode=DR,
                        )
                for m in range(MC):
                    act_r(t, m, psl[m])

            # ---- Phase U: u gates (cols 16..31), mixed fp8/bf16 ----------
            for u in range(NT_C):
                t = NT_C + u
                w8 = None
                if n8u:
                    w8 = wp.tile([P, n8u, P], FP8, tag="wu8", name=f"wu8_{u}")
                    (nc.sync if u % 2 == 0 else nc.scalar).dma_start(
                        w8[:], w_u8[u, :, :, :]
                    )
                wb = wp.tile([P, nbu, P], BF16, tag="wub", name=f"wub_{u}")
                (nc.scalar if u % 2 == 0 else nc.sync).dma_start(
                    wb[:], w_ub[u, :, :, :]
                )
                psl = [
                    pp.tile([P, M_CHUNK], F32, tag="psum", name=f"psu{t}_{m}")
                    for m in range(MC)
                ]
                n_mm = n8u // 2 + nbu  # total accumulation steps per m
                step = 0
                # fp8 DoubleRow part: x pairs then h pairs
                for q in range(N8_XU // 2):
                    for m in range(MC):
                        ms = slice(m * M_CHUNK, (m + 1) * M_CHUNK)
                        nc.tensor.matmul(
                            psl[m][:],
                            w8[:, 2 * q:2 * q + 2, :],
                            x8_pairs[q][:, 0:2, ms],
                            start=(step == 0),
                            stop=(step == n_mm - 1),
                            perf_mode=DR,
                        )
                    step += 1
                for q in range(N8_HU // 2):
                    off = N8_XU + 2 * q
                    for m in range(MC):
                        ms = slice(m * M_CHUNK, (m + 1) * M_CHUNK)
                        nc.tensor.matmul(
                            psl[m][:],
                            w8[:, off:off + 2, :],
                            h8_pairs[q][:, 0:2, ms],
                            start=(step == 0),
                            stop=(step == n_mm - 1),
                            perf_mode=DR,
                        )
                    step += 1
                # bf16 part: x k-tiles N8_XU..15, then h k-tiles N8_HU..15
                for i, src in enumerate(
                    [xb_tiles[j] for j in range(N8_XU, KT_X)]
                    + [hb_tiles[j] for j in range(N8_HU, KT_H)]
                ):
                    for m in range(MC):
                        ms = slice(m * M_CHUNK, (m + 1) * M_CHUNK)
                        nc.tensor.matmul(
                            psl[m][:],
                            wb[:, i, :],
                            src[:, ms],
                            start=(step == 0),
                            stop=(step == n_mm - 1),
                        )
                    step += 1
                for m in range(MC):
                    act_u(t, m, psl[m])

            # ---- Phase C: candidate + output combine ---------------------
            # psum = (r*h)@Wh3 (fp8 DR) + x@Wi3 (bf16);
            # h_t = u * (h - cand) + cand
            def cand_epilogue(t, m, mw, ps):
                ms = slice(m * mw, (m + 1) * mw)
                cand = sp.tile([P, mw], F32, tag="cand", name=f"c{t}_{m}")
                nc.scalar.activation(
                    cand[:], ps[:], TANH, bias=bc_all[:, t:t + 1], scale=S_INV
                )
                d = sp.tile([P, mw], F32, tag="d", name=f"d{t}_{m}")
                nc.vector.tensor_sub(d[:], hb_tiles[t][:, ms], cand[:])
                d2 = sp.tile([P, mw], F32, tag="d2", name=f"d2{t}_{m}")
                nc.vector.tensor_mul(d2[:], u_tiles[t][:, ms], d[:])
                ht = sp.tile([P, mw], F32, tag="ht", name=f"ht{t}_{m}")
                nc.vector.tensor_add(ht[:], d2[:], cand[:])
                # Outs split across both rings; tile t+1's slab DMAs are
                # issued BEFORE these in program order, so outputs never
                # delay the weight stream (run-2's 13us tail) and don't
                # drain on the slow SWDGE queue (run-3's 17us tail).
                (nc.sync if m == 0 else nc.scalar).dma_start(
                    out[t, :, ms], ht[:]
                )

            def cand_slabs(t):
                w8 = wp.tile([P, KT_H, P], FP8, tag="wc8", name=f"wc8_{t}")
                (nc.sync if t % 2 == 0 else nc.scalar).dma_start(
                    w8[:], w_c8[t, :, :, :]
                )
                wb = wp.tile([P, KT_X, P], BF16, tag="wcb", name=f"wcb_{t}")
                (nc.scalar if t % 2 == 0 else nc.sync).dma_start(
                    wb[:], w_cb[t, :, :, :]
                )
                return w8, wb

            def cand_accum(w8, wb, ps_of_m, mws):
                """Accumulate candidate GEMM for the m-chunks in ps_of_m
                (dict m -> psum tile), chunk width mws."""
                n_mm = KT_H // 2 + KT_X
                step = 0
                for q in range(KT_H // 2):
                    for m, ps in ps_of_m.items():
                        ms = slice(m * mws, (m + 1) * mws)
                        nc.tensor.matmul(
                            ps[:],
                            w8[:, 2 * q:2 * q + 2, :],
                            rh8_pairs[q][:, 0:2, ms],
                            start=(step == 0),
                            stop=(step == n_mm - 1),
                            perf_mode=DR,
                        )
                    step += 1
                for j in range(KT_X):
                    for m, ps in ps_of_m.items():
                        ms = slice(m * mws, (m + 1) * mws)
                        nc.tensor.matmul(
                            ps[:],
                            wb[:, j, :],
                            xb_tiles[j][:, ms],
                            start=(step == 0),
                            stop=(step == n_mm - 1),
                        )
                    step += 1

            # No taper on the last tile: narrow (256-wide) chunks are
            # LDWEIGHTS-bound (~305ns/matmul regardless of width), so the
            # baseline's taper costs more PE time than the epilogue tail
            # it hides.
            slabs = {0: cand_slabs(0)}
            for t in range(NT_C):
                if t + 1 < NT_C:
                    slabs[t + 1] = cand_slabs(t + 1)
                w8, wb = slabs.pop(t)
                psl = {
                    m: pp.tile(
                        [P, M_CHUNK], F32, tag="psum", name=f"psc{t}_{m}"
                    )
                    for m in range(MC)
                }
                cand_accum(w8, wb, psl, M_CHUNK)
                for m in range(MC):
                    cand_epilogue(t, m, M_CHUNK, psl[m])

    nc.compile()
    return nc


def _get_nc():
    global _CACHED_NC
    if _CACHED_NC is None:
        _CACHED_NC = _build()
    return _CACHED_NC


def _ct_blocks(w):
    """[K, N] -> [N/128 col-tiles, K/128 k-tiles, 128p, 128c] blocks."""
    K, N = w.shape
    return np.ascontiguousarray(
        w.reshape(K // P, P, N // P, P).transpose(2, 0, 1, 3)
    )


def _slab(blocks, ct, sel, np_dtype):
    """Pack k-tiles `sel` of col-tile ct into [128p, len(sel), 128c]."""
    a = blocks[ct][sel]  # [nkt, 128p, 128c]
    return np.ascontiguousarray(a.transpose(1, 0, 2)).astype(np_dtype)


def kernel(x_t, h_tm1, input_weight, hidden_state_weight, bias):
    x_t = np.asarray(x_t, dtype=np.float32)
    h_tm1 = np.asarray(h_tm1, dtype=np.float32)
    input_weight = np.asarray(input_weight, dtype=np.float32)
    hidden_state_weight = np.asarray(hidden_state_weight, dtype=np.float32)
    bias = np.asarray(bias, dtype=np.float32)

    u = UNITS
    # Gate weights: [x; h] @ [Wi[:, :2u]; Wh[:, :2u]], pre-scaled by S_W
    w_gate = np.concatenate(
        [input_weight[:, : 2 * u], hidden_state_weight[:, : 2 * u]], axis=0
    ) * np.float32(S_W)  # [4096, 4096]
    w_cand = np.concatenate(
        [input_weight[:, 2 * u:], hidden_state_weight[:, 2 * u:]], axis=0
    ) * np.float32(S_W)  # [4096, 2048]

    bg = _ct_blocks(w_gate)   # [32 ct, 32 kt, 128, 128]
    bc = _ct_blocks(w_cand)   # [16 ct, 32 kt, 128, 128]

    kt_all = list(range(KT))
    sel_u8 = list(range(N8_XU)) + list(range(KT_X, KT_X + N8_HU))
    sel_ub = list(range(N8_XU, KT_X)) + list(range(KT_X + N8_HU, KT))
    sel_c8 = list(range(KT_X, KT))      # rh k-tiles (h rows of w_cand)
    sel_cb = list(range(KT_X))          # x k-tiles

    w_r8_np = np.stack([_slab(bg, t, kt_all, NP_FP8) for t in range(NT_C)])
    w_u8_np = np.stack(
        [_slab(bg, NT_C + t, sel_u8, NP_FP8) for t in range(NT_C)]
    )
    w_ub_np = np.stack(
        [_slab(bg, NT_C + t, sel_ub, NP_BF16) for t in range(NT_C)]
    )
    w_c8_np = np.stack([_slab(bc, t, sel_c8, NP_FP8) for t in range(NT_C)])
    w_cb_np = np.stack([_slab(bc, t, sel_cb, NP_BF16) for t in range(NT_C)])

    b_g_np = np.ascontiguousarray(
        bias[: 2 * u].reshape(NT_G, P).T, dtype=np.float32
    )
    b_c_np = np.ascontiguousarray(
        bias[2 * u:].reshape(NT_C, P).T, dtype=np.float32
    )

    in_maps = []
    for i in range(N_CORES):
        sl = slice(i * B_LOC, (i + 1) * B_LOC)
        xT = x_t[sl].T  # [2048, 1024] fp32
        hT = h_tm1[sl].T
        in_maps.append(
            {
                "x8": np.ascontiguousarray(
                    xT.astype(NP_FP8).reshape(KT_X, P, B_LOC)
                ),
                "h8": np.ascontiguousarray(
                    hT.astype(NP_FP8).reshape(KT_H, P, B_LOC)
                ),
                "xb": np.ascontiguousarray(
                    xT.astype(NP_BF16).reshape(KT_X, P, B_LOC)
                ),
                "hb": np.ascontiguousarray(
                    hT.astype(NP_BF16).reshape(KT_H, P, B_LOC)
                ),
                "w_r8": w_r8_np,
                "w_u8": w_u8_np,
                "w_ub": w_ub_np,
                "w_c8": w_c8_np,
                "w_cb": w_cb_np,
                "b_g": b_g_np,
                "b_c": b_c_np,
            }
        )

    nc = _get_nc()
    res = run_bass_kernel_spmd(
        nc, in_maps, core_ids=list(range(N_CORES)), trace=TRACE
    )
    global LAST_RESULTS
    LAST_RESULTS = res

    h_t = np.empty((BATCH, UNITS), dtype=np.float32)
    for i in range(N_CORES):
        o = np.asarray(res.results[i]["out"], dtype=np.float32)
        h_t[i * B_LOC:(i + 1) * B_LOC] = o.reshape(UNITS, B_LOC).T
    return h_t


# revision 25
# speedup vs baseline: 1.5210x; 1.0041x over previous
"""GRU cell (AnotherGRUCell) on 8 TRN2 NeuronCores.

Strategy: pure data-parallel over batch (8192 rows -> 1024 rows/core),
weights replicated. No collectives.

All on-chip compute is in TRANSPOSED layout (units on the partition axis,
batch on the free axis), as in the bf16 baseline. New here: most of the
GEMM work runs in fp8-e4m3 with perf_mode=DoubleRow (2 k-tiles per PE
instruction), which roughly doubles PE matmul throughput. fp8
quantization noise is steered by a per-unit precision config chosen via
a host-side numpy simulation against the 2e-2 rel-err gate:

  - r gates (cols 0..15):     x@Wr + h@Wr fully fp8 (error is attenuated
                              through r*h -> cand -> (1-u) weighting)
  - u gates (cols 16..31):    first N8_XU/N8_HU k-tiles of x/h in fp8,
                              rest bf16 (u multiplies h directly in the
                              output, so u noise is expensive)
  - cand (r*h)@Wh3:           fully fp8 (attenuated like r)
  - cand x@Wi3:               bf16 (tanh pre-act noise is expensive)

All weights (both dtypes) are pre-scaled by S_W=32 on the host so fp8
sees a ~unit-std distribution, and every PSUM accumulation has one
uniform scale that is divided out for free inside the ScalarE
activation (out = sigmoid/tanh(psum * 1/S_W + bias)).

fp8 x/h/rh operands live in PAIR tiles [128, 2, 1024] so each DoubleRow
matmul gets its required 3D AP [128, 2, free] (pair-dim step % 16 == 0)
while startup DMAs keep per-pair dependency granularity.
"""

import numpy as np
import ml_dtypes

import concourse.bacc as bacc
import concourse.tile as tile
import concourse.mybir as mybir
from concourse.bass_utils import run_bass_kernel_spmd

N_CORES = 8
UNITS = 2048
IN_DIM = 2048
BATCH = 8192
B_LOC = BATCH // N_CORES  # 1024 batch rows per core

P = 128
KT_X = IN_DIM // P           # 16 k-tiles of x
KT_H = UNITS // P            # 16 k-tiles of h
KT = KT_X + KT_H             # 32 contraction k-tiles for [x; h]
NT_G = (2 * UNITS) // P      # 32 gate col-tiles (r: 0..15, u: 16..31)
NT_C = UNITS // P            # 16 candidate col-tiles
M_CHUNK = 512
MC = B_LOC // M_CHUNK        # 2 moving chunks per core

# Precision config: number of leading k-tiles (of 16) computed in fp8
# DoubleRow for the u-gate x/h operands. r gates and (r*h)@Wh3 are fully
# fp8; the candidate x@Wi3 is fully bf16. Must be even (DoubleRow pairs).
# Host-sim predicted rel err (matches HW to ~5 digits): 4 -> 1.571e-2,
# 8 -> 1.811e-2, 10 -> 1.920e-2 against the 2e-2 gate.
N8_XU = 10
N8_HU = 10
S_W = 32.0
S_INV = float(1.0 / S_W)

BF16 = mybir.dt.bfloat16
F32 = mybir.dt.float32
FP8 = mybir.dt.float8e4
NP_BF16 = ml_dtypes.bfloat16
NP_FP8 = ml_dtypes.float8_e4m3  # IEEE-style e4m3, max 240 == TRN FP8_EXP4
DR = mybir.MatmulPerfMode.DoubleRow

_CACHED_NC = None

# test.py sets TRACE=True to capture the NTFF profile (exec_time_ns +
# perfetto trace); the graded path leaves it off. LAST_RESULTS holds the
# BassKernelResults of the most recent run.
TRACE = False
LAST_RESULTS = None


def _build():
    nc = bacc.Bacc("TRN2", target_bir_lowering=False, debug=False)

    # fp8 transposed inputs, one [128, 1024] slice per k-tile
    x8d = nc.dram_tensor("x8", [KT_X, P, B_LOC], FP8, kind="ExternalInput")
    h8d = nc.dram_tensor("h8", [KT_H, P, B_LOC], FP8, kind="ExternalInput")
    # bf16 transposed inputs (u-gate bf16 part, cand x part, epilogues)
    xbd = nc.dram_tensor("xb", [KT_X, P, B_LOC], BF16, kind="ExternalInput")
    hbd = nc.dram_tensor("hb", [KT_H, P, B_LOC], BF16, kind="ExternalInput")
    # weights, pre-scaled by S_W, packed per col-tile as [128p, nkt, 128c]
    n8u = N8_XU + N8_HU
    nbu = KT - n8u
    w_r8 = nc.dram_tensor("w_r8", [NT_C, P, KT, P], FP8, kind="ExternalInput")
    w_u8 = nc.dram_tensor("w_u8", [NT_C, P, n8u, P], FP8, kind="ExternalInput")
    w_ub = nc.dram_tensor("w_ub", [NT_C, P, nbu, P], BF16, kind="ExternalInput")
    w_c8 = nc.dram_tensor("w_c8", [NT_C, P, KT_H, P], FP8, kind="ExternalInput")
    w_cb = nc.dram_tensor("w_cb", [NT_C, P, KT_X, P], BF16, kind="ExternalInput")
    # biases transposed: one [128, n_tiles] tensor per gate set -> 1 DMA each
    b_g = nc.dram_tensor("b_g", [P, NT_G], F32, kind="ExternalInput")
    b_c = nc.dram_tensor("b_c", [P, NT_C], F32, kind="ExternalInput")
    out = nc.dram_tensor("out", [NT_C, P, B_LOC], F32, kind="ExternalOutput")

    SIG = mybir.ActivationFunctionType.Sigmoid
    TANH = mybir.ActivationFunctionType.Tanh

    NPAIR_X = KT_X // 2
    NPAIR_H = KT_H // 2

    with tile.TileContext(nc) as tc:
        with (
            tc.tile_pool(name="resident", bufs=1) as res,
            tc.tile_pool(name="wslab", bufs=2) as wp,
            tc.tile_pool(name="psum", bufs=8, space="PSUM") as pp,
            tc.tile_pool(name="stage", bufs=2) as sp,
            tc.tile_pool(name="bias", bufs=1) as bp,
        ):
            # fp8 pair tiles: [128, 2, 1024]; pair q holds k-tiles 2q, 2q+1
            x8_pairs = [
                res.tile([P, 2, B_LOC], FP8, tag=f"x8{q}", name=f"x8{q}")
                for q in range(NPAIR_X)
            ]
            h8_pairs = [
                res.tile([P, 2, B_LOC], FP8, tag=f"h8{q}", name=f"h8{q}")
                for q in range(NPAIR_H)
            ]
            rh8_pairs = [
                res.tile([P, 2, B_LOC], FP8, tag=f"rh{q}", name=f"rh{q}")
                for q in range(NPAIR_H)
            ]
            # bf16 per-k-tile tiles
            xb_tiles = [
                res.tile([P, B_LOC], BF16, tag=f"xb{j}", name=f"xb{j}")
                for j in range(KT_X)
            ]
            hb_tiles = [
                res.tile([P, B_LOC], BF16, tag=f"hb{j}", name=f"hb{j}")
                for j in range(KT_H)
            ]
            # u gates are transient: phases U and C are interleaved per
            # col-tile, so u lives only from its sigmoid to the combine a
            # few us later (saves 30KB/partition of SBUF, spent on deeper
            # weight-slab prefetch and a wider startup interleave).

            # PE warm-up: the HAM clock gate holds the PE at 1.2 GHz until
            # it has been busy ~3.4us; fill the pre-first-matmul window
            # with dummy matmuls so the PE is un-throttled when real data
            # lands (same trick as the bf16 baseline).
            warm_src = sp.tile(
                [P, M_CHUNK], BF16, tag="warm", name="warm_src", bufs=1
            )
            nc.gpsimd.memset(warm_src[:], 0.0)
            warm_ps = pp.tile([P, M_CHUNK], F32, tag="psum", name="warm_ps")
            for w in range(8):
                nc.tensor.matmul(
                    warm_ps[:],
                    warm_src[:, :P],
                    warm_src[:],
                    start=(w == 0),
                    stop=(w == 7),
                )

            # Startup DMAs in exact consumption order of the first r-gate
            # col-tile pair, interleaved across both HWDGE rings.
            # Graduated chunk sizes (in k-tiles over the 32-long [x; h]
            # sequence); all chunk boundaries are even so DoubleRow pairs
            # never straddle a chunk.
            CHUNKS = [2, 6, 8, 8, 8]
            CB = [0, 2, 8, 16, 24, 32]  # chunk k-tile boundaries
            NT0 = 4  # r col-tiles in the startup block-interleave
            ws_first = [[None] * len(CHUNKS) for _ in range(NT0)]  # [t][chunk]
            src_dma = {}  # k-slot -> (engine, dst ap, src ap)
            for j in range(KT_X):
                eng = nc.sync if j % 2 == 0 else nc.scalar
                src_dma[j] = (eng, x8_pairs[j // 2][:, j % 2, :], x8d[j, :, :])
            for j in range(KT_H):
                eng = nc.scalar if j % 2 == 0 else nc.sync
                src_dma[KT_X + j] = (
                    eng, h8_pairs[j // 2][:, j % 2, :], h8d[j, :, :]
                )
            pre_ws = {}
            for c, cw in enumerate(CHUNKS):
                if c == 0:
                    # The very first matmul's operands go FIRST in each
                    # ring queue: x8 pair 0 then the first weight chunk.
                    for j in range(CB[0], CB[1]):
                        eng, dst, src = src_dma[j]
                        eng.dma_start(dst, src)
                if c == len(CHUNKS) - 1:
                    # Sneak the first steady-state r slabs in ahead of
                    # the last startup chunk: t=NT0's slab gates the PE
                    # right after the interleaved block and must not sit
                    # behind the bf16 input stream.
                    for t in (NT0, NT0 + 1):
                        ws = wp.tile([P, KT, P], FP8, tag="wr", name=f"wr{t}", bufs=3)
                        (nc.sync if t % 2 == 0 else nc.scalar).dma_start(
                            ws[:], w_r8[t, :, :, :]
                        )
                        pre_ws[t] = ws
                for t in range(NT0):
                    w = wp.tile(
                        [P, cw, P], FP8, tag=f"wr{t}_{c}", name=f"wr{t}_{c}",
                        bufs=1,
                    )
                    (nc.sync if t % 2 == 0 else nc.scalar).dma_start(
                        w[:], w_r8[t, :, CB[c]:CB[c + 1], :]
                    )
                    ws_first[t][c] = w
                if c > 0:
                    for j in range(CB[c], CB[c + 1]):
                        eng, dst, src = src_dma[j]
                        eng.dma_start(dst, src)

            # Biases + the early bf16 h tiles (needed by the first r
            # epilogues ~30us in) go on the SWDGE queue: the two HWDGE
            # rings deliver ~100GB/s each and are fully booked with the
            # startup x8/h8/weight traffic that gates the PE.
            bg_all = bp.tile([P, NT_G], F32, tag="bg", name="bg_all")
            nc.gpsimd.dma_start(bg_all[:], b_g[:, :])
            bc_all = bp.tile([P, NT_C], F32, tag="bc", name="bc_all")
            nc.gpsimd.dma_start(bc_all[:], b_c[:, :])
            for j in range(NT0 + 2):
                nc.gpsimd.dma_start(hb_tiles[j][:], hbd[j, :, :])

            all_pairs = x8_pairs + h8_pairs  # 16 fp8 pair tiles = 32 k-tiles

            def act_r(t, m, ps):
                """r epilogue: rh8[t] = sigmoid(ps/S_W + b) * h  (fp8)."""
                ms = slice(m * M_CHUNK, (m + 1) * M_CHUNK)
                rt = sp.tile([P, M_CHUNK], BF16, tag="rtmp", name=f"r{t}_{m}")
                nc.scalar.activation(
                    rt[:], ps[:], SIG, bias=bg_all[:, t:t + 1], scale=S_INV
                )
                nc.vector.tensor_mul(
                    rh8_pairs[t // 2][:, t % 2, ms], rt[:], hb_tiles[t][:, ms]
                )

            # ---- Phase R: r gates (cols 0..15), fully fp8 DoubleRow ------
            # The first NT0 col-tiles are block-interleaved over the
            # startup chunks (NT0*2 psum groups): the startup is input-
            # bandwidth-bound (~6MB before steady state), so the PE needs
            # ~NT0 tiles of matmul work per arriving chunk to stay busy.
            t0_groups = [(t, m) for t in range(NT0) for m in range(MC)]
            pss0 = [
                pp.tile([P, M_CHUNK], F32, tag="psum", name=f"psg0_{i}")
                for i in range(len(t0_groups))
            ]
            for c in range(len(CHUNKS)):
                q0, q1 = CB[c] // 2, CB[c + 1] // 2
                for i, (t, m) in enumerate(t0_groups):
                    ms = slice(m * M_CHUNK, (m + 1) * M_CHUNK)
                    for qq in range(q0, q1):
                        jj = qq - q0  # pair index within this chunk's slab
                        nc.tensor.matmul(
                            pss0[i][:],
                            ws_first[t][c][:, 2 * jj:2 * jj + 2, :],
                            all_pairs[qq][:, 0:2, ms],
                            start=(qq == 0),
                            stop=(qq == KT // 2 - 1),
                            perf_mode=DR,
                        )
            for i, (t, m) in enumerate(t0_groups):
                act_r(t, m, pss0[i])

            # Steady-state r cols: one fp8 slab [128, 32, 128] per col-tile,
            # m-interleaved so consecutive matmuls share the stationary
            # weight pair (one 256-col LDWEIGHTS per 2 matmuls).
            for t in range(NT0, NT_C):
                if t in pre_ws:
                    ws = pre_ws[t]
                else:
                    ws = wp.tile([P, KT, P], FP8, tag="wr", name=f"wr{t}", bufs=3)
                    (nc.sync if t % 2 == 0 else nc.scalar).dma_start(
                        ws[:], w_r8[t, :, :, :]
                    )
                # pace the bf16 inputs behind the slab they follow:
                # hb[t] lands ~1 col-tile before its epilogue needs it,
                # xb streams in over the back half of the r phase (it is
                # first read in the fused u/cand phase).
                if t < KT_H - 2:
                    (nc.scalar if t % 2 == 0 else nc.sync).dma_start(
                        hb_tiles[t + 2][:], hbd[t + 2, :, :]
                    )
                if t >= 8:
                    j0 = 2 * (t - 8)
                    (nc.scalar if t % 2 == 0 else nc.sync).dma_start(
                        xb_tiles[j0][:], xbd[j0, :, :]
                    )
                    (nc.sync if t % 2 == 0 else nc.scalar).dma_start(
                        xb_tiles[j0 + 1][:], xbd[j0 + 1, :, :]
                    )
                psl = [
                    pp.tile([P, M_CHUNK], F32, tag="psum", name=f"psr{t}_{m}")
                    for m in range(MC)
                ]
                for q in range(KT // 2):
                    for m in range(MC):
                        ms = slice(m * M_CHUNK, (m + 1) * M_CHUNK)
                        nc.tensor.matmul(
                            psl[m][:],
                            ws[:, 2 * q:2 * q + 2, :],
                            all_pairs[q][:, 0:2, ms],
                            start=(q == 0),
                            stop=(q == KT // 2 - 1),
                            perf_mode=DR,
                        )
                for m in range(MC):
                    act_r(t, m, psl[m])

            # ---- Fused phase U+C: per col-tile t, compute the u gate
            # (cols 16+t, mixed fp8/bf16) and immediately the candidate +
            # output combine for the same t. u_t lives only a few us in a
            # rotating stage tile instead of 32KB of resident SBUF.
            # psum_c = (r*h)@Wh3 (fp8 DR) + x@Wi3 (bf16);
            # h_t = u * (h - cand) + cand
            def uc_slabs(t):
                wu8t = wp.tile(
                    [P, n8u, P], FP8, tag="wu8", name=f"wu8_{t}", bufs=3
                )
                (nc.sync if t % 2 == 0 else nc.scalar).dma_start(
                    wu8t[:], w_u8[t, :, :, :]
                )
                wubt = wp.tile(
                    [P, nbu, P], BF16, tag="wub", name=f"wub_{t}", bufs=3
                )
                (nc.scalar if t % 2 == 0 else nc.sync).dma_start(
                    wubt[:], w_ub[t, :, :, :]
                )
                wc8t = wp.tile(
                    [P, KT_H, P], FP8, tag="wc8", name=f"wc8_{t}", bufs=3
                )
                (nc.sync if t % 2 == 0 else nc.scalar).dma_start(
                    wc8t[:], w_c8[t, :, :, :]
                )
                wcbt = wp.tile(
                    [P, KT_X, P], BF16, tag="wcb", name=f"wcb_{t}", bufs=3
                )
                (nc.scalar if t % 2 == 0 else nc.sync).dma_start(
                    wcbt[:], w_cb[t, :, :, :]
                )
                return wu8t, wubt, wc8t, wcbt

            def u_accum(w8, wb, psl):
                n_mm = n8u // 2 + nbu  # accumulation steps per m-chunk
                step = 0
                for q in range(N8_XU // 2):
                    for m in range(MC):
                        ms = slice(m * M_CHUNK, (m + 1) * M_CHUNK)
                        nc.tensor.matmul(
                            psl[m][:],
                            w8[:, 2 * q:2 * q + 2, :],
                            x8_pairs[q][:, 0:2, ms],
                            start=(step == 0),
                            stop=(step == n_mm - 1),
                            perf_mode=DR,
                        )
                    step += 1
                for q in range(N8_HU // 2):
                    off = N8_XU + 2 * q
                    for m in range(MC):
                        ms = slice(m * M_CHUNK, (m + 1) * M_CHUNK)
                        nc.tensor.matmul(
                            psl[m][:],
                            w8[:, off:off + 2, :],
                            h8_pairs[q][:, 0:2, ms],
                            start=(step == 0),
                            stop=(step == n_mm - 1),
                            perf_mode=DR,
                        )
                    step += 1
                # bf16 part: x k-tiles N8_XU..15, then h k-tiles N8_HU..15
                for i, src in enumerate(
                    [xb_tiles[j] for j in range(N8_XU, KT_X)]
                    + [hb_tiles[j] for j in range(N8_HU, KT_H)]
                ):
                    for m in range(MC):
                        ms = slice(m * M_CHUNK, (m + 1) * M_CHUNK)
                        nc.tensor.matmul(
                            psl[m][:],
                            wb[:, i, :],
                            src[:, ms],
                            start=(step == 0),
                            stop=(step == n_mm - 1),
                        )
                    step += 1

            def cand_accum(w8, wb, psl):
                n_mm = KT_H // 2 + KT_X
                step = 0
                for q in range(KT_H // 2):
                    for m in range(MC):
                        ms = slice(m * M_CHUNK, (m + 1) * M_CHUNK)
                        nc.tensor.matmul(
                            psl[m][:],
                            w8[:, 2 * q:2 * q + 2, :],
                            rh8_pairs[q][:, 0:2, ms],
                            start=(step == 0),
                            stop=(step == n_mm - 1),
                            perf_mode=DR,
                        )
                    step += 1
                for j in range(KT_X):
                    for m in range(MC):
                        ms = slice(m * M_CHUNK, (m + 1) * M_CHUNK)
                        nc.tensor.matmul(
                            psl[m][:],
                            wb[:, j, :],
                            xb_tiles[j][:, ms],
                            start=(step == 0),
                            stop=(step == n_mm - 1),
                        )
                    step += 1

            def cand_epilogue(t, m, ut, ps):
                ms = slice(m * M_CHUNK, (m + 1) * M_CHUNK)
                cand = sp.tile([P, M_CHUNK], F32, tag="cand", name=f"c{t}_{m}")
                nc.scalar.activation(
                    cand[:], ps[:], TANH, bias=bc_all[:, t:t + 1], scale=S_INV
                )
                d = sp.tile([P, M_CHUNK], F32, tag="d", name=f"d{t}_{m}")
                nc.vector.tensor_sub(d[:], hb_tiles[t][:, ms], cand[:])
                d2 = sp.tile([P, M_CHUNK], F32, tag="d2", name=f"d2{t}_{m}")
                nc.vector.tensor_mul(d2[:], ut[:, ms], d[:])
                ht = sp.tile([P, M_CHUNK], F32, tag="ht", name=f"ht{t}_{m}")
                nc.vector.tensor_add(ht[:], d2[:], cand[:])
                # Outs split across both rings; tile t+1's slab DMAs are
                # issued BEFORE these in program order, so outputs never
                # delay the weight stream (run-2's 13us tail) and don't
                # drain on the slow SWDGE queue (run-3's 17us tail).
                (nc.sync if m == 0 else nc.scalar).dma_start(
                    out[t, :, ms], ht[:]
                )

            slabs = {0: uc_slabs(0)}
            for t in range(NT_C):
                if t + 1 < NT_C:
                    slabs[t + 1] = uc_slabs(t + 1)
                wu8t, wubt, wc8t, wcbt = slabs.pop(t)
                ut = sp.tile([P, B_LOC], BF16, tag="ut", name=f"ut{t}")
                psu = [
                    pp.tile([P, M_CHUNK], F32, tag="psum", name=f"psu{t}_{m}")
                    for m in range(MC)
                ]
                u_accum(wu8t, wubt, psu)
                for m in range(MC):
                    ms = slice(m * M_CHUNK, (m + 1) * M_CHUNK)
                    nc.scalar.activation(
                        ut[:, ms], psu[m][:], SIG,
                        bias=bg_all[:, NT_C + t:NT_C + t + 1], scale=S_INV,
                    )
                psc = [
                    pp.tile([P, M_CHUNK], F32, tag="psum", name=f"psc{t}_{m}")
                    for m in range(MC)
                ]
                cand_accum(wc8t, wcbt, psc)
                for m in range(MC):
                    cand_epilogue(t, m, ut, psc[m])

    nc.compile()
    return nc


def _get_nc():
    global _CACHED_NC
    if _CACHED_NC is None:
        _CACHED_NC = _build()
    return _CACHED_NC


def _ct_blocks(w):
    """[K, N] -> [N/128 col-tiles, K/128 k-tiles, 128p, 128c] blocks."""
    K, N = w.shape
    return np.ascontiguousarray(
        w.reshape(K // P, P, N // P, P).transpose(2, 0, 1, 3)
    )


def _slab(blocks, ct, sel, np_dtype):
    """Pack k-tiles `sel` of col-tile ct into [128p, len(sel), 128c]."""
    a = blocks[ct][sel]  # [nkt, 128p, 128c]
    return np.ascontiguousarray(a.transpose(1, 0, 2)).astype(np_dtype)


def kernel(x_t, h_tm1, input_weight, hidden_state_weight, bias):
    x_t = np.asarray(x_t, dtype=np.float32)
    h_tm1 = np.asarray(h_tm1, dtype=np.float32)
    input_weight = np.asarray(input_weight, dtype=np.float32)
    hidden_state_weight = np.asarray(hidden_state_weight, dtype=np.float32)
    bias = np.asarray(bias, dtype=np.float32)

    u = UNITS
    # Gate weights: [x; h] @ [Wi[:, :2u]; Wh[:, :2u]], pre-scaled by S_W
    w_gate = np.concatenate(
        [input_weight[:, : 2 * u], hidden_state_weight[:, : 2 * u]], axis=0
    ) * np.float32(S_W)  # [4096, 4096]
    w_cand = np.concatenate(
        [input_weight[:, 2 * u:], hidden_state_weight[:, 2 * u:]], axis=0
    ) * np.float32(S_W)  # [4096, 2048]

    bg = _ct_blocks(w_gate)   # [32 ct, 32 kt, 128, 128]
    bc = _ct_blocks(w_cand)   # [16 ct, 32 kt, 128, 128]

    kt_all = list(range(KT))
    sel_u8 = list(range(N8_XU)) + list(range(KT_X, KT_X + N8_HU))
    sel_ub = list(range(N8_XU, KT_X)) + list(range(KT_X + N8_HU, KT))
    sel_c8 = list(range(KT_X, KT))      # rh k-tiles (h rows of w_cand)
    sel_cb = list(range(KT_X))          # x k-tiles

    w_r8_np = np.stack([_slab(bg, t, kt_all, NP_FP8) for t in range(NT_C)])
    w_u8_np = np.stack(
        [_slab(bg, NT_C + t, sel_u8, NP_FP8) for t in range(NT_C)]
    )
    w_ub_np = np.stack(
        [_slab(bg, NT_C + t, sel_ub, NP_BF16) for t in range(NT_C)]
    )
    w_c8_np = np.stack([_slab(bc, t, sel_c8, NP_FP8) for t in range(NT_C)])
    w_cb_np = np.stack([_slab(bc, t, sel_cb, NP_BF16) for t in range(NT_C)])

    b_g_np = np.ascontiguousarray(
        bias[: 2 * u].reshape(NT_G, P).T, dtype=np.float32
    )
    b_c_np = np.ascontiguousarray(
        bias[2 * u:].reshape(NT_C, P).T, dtype=np.float32
    )

    in_maps = []
    for i in range(N_CORES):
        sl = slice(i * B_LOC, (i + 1) * B_LOC)
        xT = x_t[sl].T  # [2048, 1024] fp32
        hT = h_tm1[sl].T
        in_maps.append(
            {
                "x8": np.ascontiguousarray(
                    xT.astype(NP_FP8).reshape(KT_X, P, B_LOC)
                ),
                "h8": np.ascontiguousarray(
                    hT.astype(NP_FP8).reshape(KT_H, P, B_LOC)
                ),
                "xb": np.ascontiguousarray(
                    xT.astype(NP_BF16).reshape(KT_X, P, B_LOC)
                ),
                "hb": np.ascontiguousarray(
                    hT.astype(NP_BF16).reshape(KT_H, P, B_LOC)
                ),
                "w_r8": w_r8_np,
                "w_u8": w_u8_np,
                "w_ub": w_ub_np,
                "w_c8": w_c8_np,
                "w_cb": w_cb_np,
                "b_g": b_g_np,
                "b_c": b_c_np,
            }
        )

    nc = _get_nc()
    res = run_bass_kernel_spmd(
        nc, in_maps, core_ids=list(range(N_CORES)), trace=TRACE
    )
    global LAST_RESULTS
    LAST_RESULTS = res

    h_t = np.empty((BATCH, UNITS), dtype=np.float32)
    for i in range(N_CORES):
        o = np.asarray(res.results[i]["out"], dtype=np.float32)
        h_t[i * B_LOC:(i + 1) * B_LOC] = o.reshape(UNITS, B_LOC).T
    return h_t
